# revision 10
# baseline (speedup 1.0000x reference)
"""Trainium2 Bass kernel for the emoji-box decoder problem (optimized v2).

Math: per picture, softmax(-d2) over emoji pixels is separable:
softmax_r (x) softmax_c.  This version postpones BOTH softmax
normalizations: it computes unnormalized ErT[i,r] = exp(-(src_r[i]-r)^2)
and EcT[j,c] = exp(-(src_c[j]-c)^2) DIRECTLY in transposed layout
(partition = emoji index), so no PE transposes are needed.  The
normalizers Zr = sum_i ErT, Zc = sum_j EcT come from ones-matmuls on the
PE, and 1/Zr, 1/Zc are folded into the box mask outer product:
    res = eR * Mz + (valid - Mv)
    eR  = ErT^T-contractions: t1[j,r] = sum_i wimg[i,(ch,j)] ErT[i,r],
          eR[(ch,r),c] = sum_j t1 EcT[j,c]
    Mz  = (valid*rowin*rzr) (x) (colin*rzc),  Mv = (valid*rowin) (x) colin
which equals where(valid, where(inside, R, 1), 0).

Emoji selection: argmax of logits -> gpsimd register -> dynamic-offset
DRAM->SBUF DMA gather of the selected emoji (bf16, i-major layout so
each partition reads one contiguous 384B chunk).

Matmuls run in bf16 (1-pass PE) — images are cast to bf16 on the host
(layout/dtype prep only), exp outputs are written as bf16.

Sharding: 8 cores = 2 pictures x 4 row-blocks of 64 canvas rows.
xmeta is host-replicated to all 128 partitions (no on-device broadcast).
"""

import sys

import numpy as np

if "/opt/trn_rl_repo" not in sys.path:
    sys.path.insert(0, "/opt/trn_rl_repo")

import ml_dtypes

import concourse.bacc as bacc
import concourse.bass as bass
import concourse.mybir as mybir
import concourse.tile as tile
from concourse.bass_utils import run_bass_kernel_spmd


def _ensure_ntff_hook():
    """The image's antenv package lacks axon_hooks, so trn_boot's NTFF
    profile hook install degrades silently and run_bass_kernel_spmd
    crashes on `from antenv.axon_hooks import ...` when trace=True.
    Provide the module and install the ctypes hook ourselves."""
    import types

    try:
        from antenv.axon_hooks import get_axon_ntff_profile_hook  # noqa: F401

        return
    except ImportError:
        pass
    mod = types.ModuleType("antenv.axon_hooks")
    _hook = [None]
    mod.set_axon_ntff_profile_hook = lambda h: _hook.__setitem__(0, h)
    mod.get_axon_ntff_profile_hook = lambda: _hook[0]
    try:
        import antenv

        sys.modules["antenv.axon_hooks"] = mod
        antenv.axon_hooks = mod
        from trn_agent_boot.trn_boot import _ntff_profile_via_ctypes

        hook = _ntff_profile_via_ctypes("/opt/axon/libaxon_pjrt.so")
        if hook is not None:
            mod.set_axon_ntff_profile_hook(hook)
    except Exception:
        pass


_ensure_ntff_hook()

F32 = mybir.dt.float32
BF16 = mybir.dt.bfloat16
I32 = mybir.dt.int32
AF = mybir.ActivationFunctionType
OP = mybir.AluOpType
AX = mybir.AxisListType

MAGIC = 8388608.0  # 2**23; x + MAGIC - MAGIC == rint(x) for 0 <= x < 2**22

N_CORES = 8
H = 256
S = 64
N_IMG = 14
RB = 64  # canvas rows per core


def build_nc():
    nc = bacc.Bacc("TRN2", target_bir_lowering=False, debug=False)

    xmeta_d = nc.dram_tensor("xmeta", [128, 24], F32, kind="ExternalInput")
    imgs_d = nc.dram_tensor("imgs", [N_IMG, S, 3 * S], BF16, kind="ExternalInput")
    out_d = nc.dram_tensor("out", [3, RB, H], F32, kind="ExternalOutput")

    with tile.TileContext(nc) as tc:
        with (
            tc.tile_pool(name="constp", bufs=1) as constp,
            tc.tile_pool(name="workp", bufs=2) as workp,
            tc.tile_pool(name="outp", bufs=1) as outp,
            tc.tile_pool(name="ps_z", bufs=1, space="PSUM") as ps_z,
            tc.tile_pool(name="ps_m", bufs=2, space="PSUM") as ps_m,
            tc.tile_pool(name="ps_t1", bufs=1, space="PSUM") as ps_t1,
            tc.tile_pool(name="ps_r", bufs=2, space="PSUM") as ps_r,
        ):
            # ---- warm the scalar-engine activation table early so the
            # ~1.3us ACT_TABLE_LOAD overlaps the input DMA
            warm = workp.tile([1, 1], F32)
            nc.gpsimd.memset(warm[:], 0.0)
            warm2 = workp.tile([1, 1], F32)
            nc.scalar.activation(warm2[:], warm[:], AF.Exp)

            # ---- input DMAs: xmeta (sync HWDGE) + all-images prefetch
            # (scalar HWDGE, separate ring) issued back-to-back at entry
            xb = constp.tile([128, 24], F32)
            nc.sync.dma_start(xb[:], xmeta_d[:])

            # ---- constants / iotas (gpsimd+vector, overlap the DMAs)
            iota_pi = constp.tile([128, 1], I32)
            nc.gpsimd.iota(iota_pi[:], pattern=[[1, 1]], base=0, channel_multiplier=1)
            iota_pf = constp.tile([128, 1], F32)
            nc.vector.tensor_copy(iota_pf[:], iota_pi[:])
            iota14i = constp.tile([1, N_IMG], I32)
            nc.gpsimd.iota(iota14i[:], pattern=[[1, N_IMG]], base=0, channel_multiplier=0)
            iota14f = constp.tile([1, N_IMG], F32)
            nc.vector.tensor_copy(iota14f[:], iota14i[:])
            iota64i = constp.tile([128, 64], I32)
            nc.gpsimd.iota(iota64i[:], pattern=[[1, 64]], base=0, channel_multiplier=0)
            iota64f = constp.tile([128, 64], F32)
            nc.vector.tensor_copy(iota64f[:], iota64i[:])
            iota256i = constp.tile([64, 256], I32)
            nc.gpsimd.iota(iota256i[:], pattern=[[1, 256]], base=0, channel_multiplier=0)
            iota256f = constp.tile([64, 256], F32)
            nc.vector.tensor_copy(iota256f[:], iota256i[:])
            c64 = constp.tile([128, 1], F32)
            nc.vector.memset(c64[:], 1.0 / 64.0)
            ones64_bf = constp.tile([64, 1], BF16)
            nc.gpsimd.memset(ones64_bf[:], 1.0)

            # ================= after xmeta arrives =================
            # ---- emoji index chain (gpsimd) -> sync register -> gather
            rmax = workp.tile([1, 1], F32)
            nc.vector.tensor_reduce(rmax[:], xb[0:1, 5:19], AX.X, OP.max)
            dotj = workp.tile([1, N_IMG], F32)
            nc.vector.scalar_tensor_tensor(
                dotj[:], xb[0:1, 5:19], rmax[:], iota14f[:], OP.is_ge, OP.mult
            )
            idxf = workp.tile([1, 1], F32)
            nc.vector.tensor_reduce(idxf[:], dotj[:], AX.X, OP.add)
            idxi = workp.tile([1, 1], I32)
            nc.vector.tensor_copy(idxi[:], idxf[:])
            wimg = constp.tile([S, 3 * S], BF16)
            with nc.gpsimd.register("ridx") as ridx:
                nc.gpsimd.reg_load(ridx, idxi[0:1, 0:1])
                off = nc.gpsimd.snap(ridx)
                nc.gpsimd.dma_start(
                    wimg[:], imgs_d[bass.ds(off, 1), :, :].squeeze(0)
                )

            # ---- rounded box coords cs = rint(256 * X[0:4]) (vector)
            cs = constp.tile([128, 4], F32)
            nc.vector.tensor_scalar(cs[:], xb[:, 0:4], 256.0, MAGIC, OP.mult, OP.add)
            nc.vector.tensor_scalar(cs[:], cs[:], MAGIC, None, OP.subtract)

            # ---- row exp table ErT[i, r] = exp(-(src_r[i] - (r0+r))^2), bf16
            boxr64 = constp.tile([128, 1], F32)
            nc.vector.scalar_tensor_tensor(
                boxr64[:], cs[:, 1:2], cs[:, 0:1], c64[:], OP.subtract, OP.mult
            )
            cs0r0 = workp.tile([128, 1], F32)
            nc.vector.tensor_tensor(cs0r0[:], cs[:, 0:1], xb[:, 19:20], OP.subtract)
            svecR = workp.tile([64, 1], F32)
            nc.vector.tensor_scalar(
                svecR[:], iota_pf[0:64, :], boxr64[0:64, :], cs0r0[0:64, :], OP.mult, OP.add
            )
            drT = workp.tile([64, 64], F32)
            nc.vector.tensor_scalar(drT[:], iota64f[0:64, :], -1.0, svecR[:], OP.mult, OP.add)
            drT2 = workp.tile([64, 64], F32)
            nc.scalar.square(drT2[:], drT[:])
            ErT = constp.tile([64, 64], BF16)
            nc.scalar.activation(ErT[:], drT2[:], AF.Exp, scale=-1.0)

            # ---- col exp table EcT[j, c] = exp(-(src_c[j] - c)^2), bf16
            boxc64 = constp.tile([128, 1], F32)
            nc.vector.scalar_tensor_tensor(
                boxc64[:], cs[:, 3:4], cs[:, 2:3], c64[:], OP.subtract, OP.mult
            )
            svecC = workp.tile([64, 1], F32)
            nc.vector.tensor_scalar(
                svecC[:], iota_pf[0:64, :], boxc64[0:64, :], cs[0:64, 2:3], OP.mult, OP.add
            )
            dcT = workp.tile([64, 256], F32)
            nc.vector.tensor_scalar(dcT[:], iota256f[:], -1.0, svecC[:], OP.mult, OP.add)
            dcT2 = workp.tile([64, 256], F32)
            nc.scalar.square(dcT2[:], dcT[:])
            EcT = constp.tile([64, 256], BF16)
            nc.scalar.activation(EcT[:], dcT2[:], AF.Exp, scale=-1.0)

            # ---- normalizers via PE ones-matmuls, shared PSUM bank
            zall_ps = ps_z.tile([1, 320], F32, tag="z")
            nc.tensor.matmul(zall_ps[0:1, 0:64], ones64_bf[:], ErT[:])
            nc.tensor.matmul(zall_ps[0:1, 64:320], ones64_bf[:], EcT[:])
            rzr = workp.tile([1, 64], F32)
            nc.vector.tensor_scalar(rzr[:], zall_ps[0:1, 0:64], 1e-30, None, OP.add)
            nc.vector.reciprocal(rzr[:], rzr[:])
            rzc = workp.tile([1, 256], F32)
            nc.vector.tensor_scalar(rzc[:], zall_ps[0:1, 64:320], 1e-30, None, OP.add)
            nc.vector.reciprocal(rzc[:], rzc[:])

            # ---- valid flag (only the non-tautological conditions;
            # inputs are sorted in [0,1] so 0<=x1<=x2<=256 always holds)
            v4 = workp.tile([128, 1], F32)
            nc.vector.tensor_tensor(v4[:], cs[:, 1:2], cs[:, 0:1], OP.is_gt)
            v5 = workp.tile([128, 1], F32)
            nc.vector.tensor_tensor(v5[:], cs[:, 3:4], cs[:, 2:3], OP.is_gt)
            valid = constp.tile([128, 1], F32)
            nc.vector.tensor_tensor(valid[:], v4[:], v5[:], OP.mult)

            # ---- box-interval rows/cols (gpsimd) for the mask outers
            cs1r0 = workp.tile([128, 1], F32)
            nc.vector.tensor_tensor(cs1r0[:], cs[:, 1:2], xb[:, 19:20], OP.subtract)
            r_ge = workp.tile([1, 64], F32)
            nc.vector.tensor_scalar(r_ge[:], iota64f[0:1, :], cs0r0[0:1, :], None, OP.is_ge)
            r_lt = workp.tile([1, 64], F32)
            nc.vector.tensor_scalar(r_lt[:], iota64f[0:1, :], cs1r0[0:1, :], None, OP.is_lt)
            rowinv = workp.tile([1, 64], F32)  # valid * inside_r
            nc.vector.scalar_tensor_tensor(
                rowinv[:], r_ge[:], valid[0:1, :], r_lt[:], OP.mult, OP.mult
            )
            c_ge = workp.tile([1, 256], F32)
            nc.vector.tensor_scalar(c_ge[:], iota256f[0:1, :], cs[0:1, 2:3], None, OP.is_ge)
            c_lt = workp.tile([1, 256], F32)
            nc.vector.tensor_scalar(c_lt[:], iota256f[0:1, :], cs[0:1, 3:4], None, OP.is_lt)
            colin = workp.tile([1, 256], F32)
            nc.vector.tensor_tensor(colin[:], c_ge[:], c_lt[:], OP.mult)

            # ---- scaled/plain mask factor rows, stacked twice for (ch0,ch1)
            rowz2 = workp.tile([1, 128], F32)
            nc.vector.tensor_tensor(rowz2[0:1, 0:64], rowinv[:], rzr[:], OP.mult)
            nc.vector.tensor_copy(rowz2[0:1, 64:128], rowz2[0:1, 0:64])
            rowv2 = workp.tile([1, 128], F32)
            nc.scalar.copy(rowv2[0:1, 0:64], rowinv[:])
            nc.scalar.copy(rowv2[0:1, 64:128], rowinv[:])
            colz = workp.tile([1, 256], F32)
            nc.vector.tensor_tensor(colz[:], colin[:], rzc[:], OP.mult)

            # ---- mask outer products on PE
            mz_ps = ps_m.tile([128, 256], F32, tag="m", name="mzps")
            nc.tensor.matmul(mz_ps[:], rowz2[:], colz[:])
            mv_ps = ps_m.tile([128, 256], F32, tag="m", name="mvps")
            nc.tensor.matmul(mv_ps[:], rowv2[:], colin[:])
            mz_sb = constp.tile([128, 256], F32)
            nc.scalar.copy(mz_sb[:], mz_ps[:])
            w2 = constp.tile([128, 256], F32)
            nc.vector.tensor_scalar(w2[:], mv_ps[:], -1.0, valid[:], OP.mult, OP.add)

            # ---- t1[ch][j, r] = sum_i wimg[i, (ch,j)] * ErT[i, r]
            t1_ps = ps_t1.tile([64, 192], F32, tag="t1")
            for ch in range(3):
                nc.tensor.matmul(
                    t1_ps[:, 64 * ch : 64 * (ch + 1)],
                    wimg[:, 64 * ch : 64 * (ch + 1)],
                    ErT[:],
                )
            t1all = constp.tile([64, 192], BF16)
            nc.scalar.copy(t1all[:, 0:64], t1_ps[:, 0:64])
            nc.vector.tensor_copy(t1all[:, 64:128], t1_ps[:, 64:128])
            nc.vector.tensor_copy(t1all[:, 128:192], t1_ps[:, 128:192])

            # ---- eR matmuls
            er_ab_ps = ps_r.tile([128, 256], F32, tag="rab", name="erab")
            nc.tensor.matmul(er_ab_ps[:], t1all[:, 0:128], EcT[:])
            er_c_ps = ps_r.tile([64, 256], F32, tag="rc", name="erc")
            nc.tensor.matmul(er_c_ps[:], t1all[:, 128:192], EcT[:])

            # ---- blend: res = eR*Mz + (valid - Mv)
            res_ab = outp.tile([128, 256], F32)
            nc.vector.tensor_tensor(res_ab[:], er_ab_ps[:], mz_sb[:], OP.mult)
            nc.vector.tensor_tensor(res_ab[:], res_ab[:], w2[:], OP.add)
            res_c = outp.tile([64, 256], F32)
            nc.vector.tensor_tensor(res_c[:], er_c_ps[:], mz_sb[0:64, :], OP.mult)
            nc.vector.tensor_tensor(res_c[:], res_c[:], w2[0:64, :], OP.add)

            # ---- output DMAs on the two HWDGE rings in parallel
            nc.sync.dma_start(
                out_d[0:2, :, :].rearrange("a b c -> (a b) c"), res_ab[:]
            )
            nc.scalar.dma_start(out_d[2, :, :], res_c[:])

    nc.compile()
    return nc


_CACHE = {}


def get_nc():
    if "nc" not in _CACHE:
        _CACHE["nc"] = build_nc()
    return _CACHE["nc"]


def make_in_maps(X, images):
    X = np.ascontiguousarray(np.asarray(X, np.float32))
    images = np.ascontiguousarray(np.asarray(images, np.float32))
    # layout/dtype prep only: [14,4,64,64] f32 -> [14, 64(i), 3*64(ch,j)] bf16
    imgs_gt = np.ascontiguousarray(
        images[:, 0:3].transpose(0, 2, 1, 3).reshape(N_IMG, S, 3 * S)
    ).astype(ml_dtypes.bfloat16)
    in_maps = []
    for c in range(N_CORES):
        pic, rb = divmod(c, 4)
        xm = np.zeros((1, 24), np.float32)
        xm[0, :19] = X[pic, 0]
        xm[0, 19] = float(RB * rb)
        in_maps.append({"xmeta": np.tile(xm, (128, 1)), "imgs": imgs_gt})
    return in_maps


def assemble(results):
    out = np.empty((2, 3, H, H), np.float32)
    for c in range(N_CORES):
        pic, rb = divmod(c, 4)
        out[pic, :, RB * rb : RB * (rb + 1), :] = results[c]["out"]
    return out


def _axon_reset():
    try:
        import ctypes

        import jax

        jax.devices()
        ctypes.CDLL("/opt/axon/libaxon_pjrt.so").axon_reset()
    except Exception:
        pass


def kernel(X, images):
    nc = get_nc()
    in_maps = make_in_maps(X, images)
    try:
        res = run_bass_kernel_spmd(nc, in_maps, list(range(N_CORES)))
    except Exception:
        # the axon terminal can be left in a bad state by earlier failed
        # runs (LoadExecutable errors); reset and retry once
        _axon_reset()
        res = run_bass_kernel_spmd(nc, in_maps, list(range(N_CORES)))
    return assemble(res.results)


# revision 14
# speedup vs baseline: 1.1978x; 1.1978x over previous
"""Trainium2 Bass kernel for the emoji-box decoder problem (optimized v2).

Math: per picture, softmax(-d2) over emoji pixels is separable:
softmax_r (x) softmax_c.  This version postpones BOTH softmax
normalizations: it computes unnormalized ErT[i,r] = exp(-(src_r[i]-r)^2)
and EcT[j,c] = exp(-(src_c[j]-c)^2) DIRECTLY in transposed layout
(partition = emoji index), so no PE transposes are needed.  The
normalizers Zr = sum_i ErT, Zc = sum_j EcT come from ones-matmuls on the
PE, and 1/Zr, 1/Zc are folded into the box mask outer product:
    res = eR * Mz + (valid - Mv)
    eR  = ErT^T-contractions: t1[j,r] = sum_i wimg[i,(ch,j)] ErT[i,r],
          eR[(ch,r),c] = sum_j t1 EcT[j,c]
    Mz  = (valid*rowin*rzr) (x) (colin*rzc),  Mv = (valid*rowin) (x) colin
which equals where(valid, where(inside, R, 1), 0).

Emoji selection: argmax of logits -> gpsimd register -> dynamic-offset
DRAM->SBUF DMA gather of the selected emoji (bf16, i-major layout so
each partition reads one contiguous 384B chunk).

Matmuls run in bf16 (1-pass PE) — images are cast to bf16 on the host
(layout/dtype prep only), exp outputs are written as bf16.

Sharding: 8 cores = 2 pictures x 4 row-blocks of 64 canvas rows.
xmeta is host-replicated to all 128 partitions (no on-device broadcast).
"""

import sys

import numpy as np

if "/opt/trn_rl_repo" not in sys.path:
    sys.path.insert(0, "/opt/trn_rl_repo")

import ml_dtypes

import concourse.bacc as bacc
import concourse.bass as bass
import concourse.mybir as mybir
import concourse.tile as tile
from concourse.bass_utils import run_bass_kernel_spmd


def _ensure_ntff_hook():
    """The image's antenv package lacks axon_hooks, so trn_boot's NTFF
    profile hook install degrades silently and run_bass_kernel_spmd
    crashes on `from antenv.axon_hooks import ...` when trace=True.
    Provide the module and install the ctypes hook ourselves."""
    import types

    try:
        from antenv.axon_hooks import get_axon_ntff_profile_hook  # noqa: F401

        return
    except ImportError:
        pass
    mod = types.ModuleType("antenv.axon_hooks")
    _hook = [None]
    mod.set_axon_ntff_profile_hook = lambda h: _hook.__setitem__(0, h)
    mod.get_axon_ntff_profile_hook = lambda: _hook[0]
    try:
        import antenv

        sys.modules["antenv.axon_hooks"] = mod
        antenv.axon_hooks = mod
        from trn_agent_boot.trn_boot import _ntff_profile_via_ctypes

        hook = _ntff_profile_via_ctypes("/opt/axon/libaxon_pjrt.so")
        if hook is not None:
            mod.set_axon_ntff_profile_hook(hook)
    except Exception:
        pass


_ensure_ntff_hook()

F32 = mybir.dt.float32
BF16 = mybir.dt.bfloat16
I32 = mybir.dt.int32
AF = mybir.ActivationFunctionType
OP = mybir.AluOpType
AX = mybir.AxisListType

MAGIC = 8388608.0  # 2**23; x + MAGIC - MAGIC == rint(x) for 0 <= x < 2**22

N_CORES = 8
H = 256
S = 64
N_IMG = 14
RB = 64  # canvas rows per core


def build_nc():
    nc = bacc.Bacc("TRN2", target_bir_lowering=False, debug=False)

    xmeta_d = nc.dram_tensor("xmeta", [128, 24], F32, kind="ExternalInput")
    imgs_d = nc.dram_tensor("imgs", [N_IMG, S, 3 * S], BF16, kind="ExternalInput")
    out_d = nc.dram_tensor("out", [3, RB, H], F32, kind="ExternalOutput")

    with tile.TileContext(nc) as tc:
        with (
            tc.tile_pool(name="constp", bufs=1) as constp,
            tc.tile_pool(name="workp", bufs=2) as workp,
            tc.tile_pool(name="outp", bufs=1) as outp,
            tc.tile_pool(name="ps_z", bufs=1, space="PSUM") as ps_z,
            tc.tile_pool(name="ps_m", bufs=2, space="PSUM") as ps_m,
            tc.tile_pool(name="ps_t1", bufs=1, space="PSUM") as ps_t1,
            tc.tile_pool(name="ps_r", bufs=2, space="PSUM") as ps_r,
        ):
            # ---- warm the scalar-engine activation table early so the
            # ~1.3us ACT_TABLE_LOAD overlaps the input DMA
            warm = workp.tile([1, 1], F32)
            nc.gpsimd.memset(warm[:], 0.0)
            warm2 = workp.tile([1, 1], F32)
            nc.scalar.activation(warm2[:], warm[:], AF.Exp)

            # ---- input DMAs: xmeta (sync HWDGE) + all-images prefetch
            # (scalar HWDGE, separate ring) issued back-to-back at entry
            xb = constp.tile([128, 24], F32)
            nc.sync.dma_start(xb[:], xmeta_d[:])

            # ---- constants / iotas (gpsimd+vector, overlap the DMAs)
            iota_pi = constp.tile([128, 1], I32)
            nc.gpsimd.iota(iota_pi[:], pattern=[[1, 1]], base=0, channel_multiplier=1)
            iota_pf = constp.tile([128, 1], F32)
            nc.vector.tensor_copy(iota_pf[:], iota_pi[:])
            iota14i = constp.tile([1, N_IMG], I32)
            nc.gpsimd.iota(iota14i[:], pattern=[[1, N_IMG]], base=0, channel_multiplier=0)
            iota14f = constp.tile([1, N_IMG], F32)
            nc.vector.tensor_copy(iota14f[:], iota14i[:])
            iota64i = constp.tile([128, 64], I32)
            nc.gpsimd.iota(iota64i[:], pattern=[[1, 64]], base=0, channel_multiplier=0)
            iota64f = constp.tile([128, 64], F32)
            nc.vector.tensor_copy(iota64f[:], iota64i[:])
            iota256i = constp.tile([64, 256], I32)
            nc.gpsimd.iota(iota256i[:], pattern=[[1, 256]], base=0, channel_multiplier=0)
            iota256f = constp.tile([64, 256], F32)
            nc.vector.tensor_copy(iota256f[:], iota256i[:])
            c64 = constp.tile([128, 1], F32)
            nc.vector.memset(c64[:], 1.0 / 64.0)
            ones64_bf = constp.tile([64, 1], BF16)
            nc.gpsimd.memset(ones64_bf[:], 1.0)
            eps1 = constp.tile([1, 1], F32)
            nc.gpsimd.memset(eps1[:], 1e-30)

            # ================= after xmeta arrives =================
            # ---- emoji index chain (gpsimd) -> sync register -> gather
            rmax = workp.tile([1, 1], F32)
            nc.vector.tensor_reduce(rmax[:], xb[0:1, 5:19], AX.X, OP.max)
            dotj = workp.tile([1, N_IMG], I32)
            nc.vector.scalar_tensor_tensor(
                dotj[:], xb[0:1, 5:19], rmax[:], iota14f[:], OP.is_ge, OP.mult
            )
            idxi = workp.tile([1, 1], I32)
            with nc.allow_low_precision(reason="argmax index sum is exact in i32"):
                nc.vector.tensor_reduce(idxi[:], dotj[:], AX.X, OP.add)
            wimg = constp.tile([S, 3 * S], BF16)
            with nc.gpsimd.register("ridx") as ridx:
                nc.gpsimd.reg_load(ridx, idxi[0:1, 0:1])
                off = nc.gpsimd.snap(ridx)
                nc.gpsimd.dma_start(
                    wimg[:], imgs_d[bass.ds(off, 1), :, :].squeeze(0)
                )

            # ---- rounded box coords cs = rint(256 * X[0:4]) (vector)
            cs = constp.tile([128, 4], F32)
            nc.vector.tensor_scalar(cs[:], xb[:, 0:4], 256.0, MAGIC, OP.mult, OP.add)
            nc.vector.tensor_scalar(cs[:], cs[:], MAGIC, None, OP.subtract)

            # ---- row exp table ErT[i, r] = exp(-(src_r[i] - (r0+r))^2), bf16
            boxr64 = constp.tile([128, 1], F32)
            nc.vector.scalar_tensor_tensor(
                boxr64[:], cs[:, 1:2], cs[:, 0:1], c64[:], OP.subtract, OP.mult
            )
            cs0r0 = workp.tile([128, 1], F32)
            nc.vector.tensor_tensor(cs0r0[:], cs[:, 0:1], xb[:, 19:20], OP.subtract)
            svecR = workp.tile([64, 1], F32)
            nc.vector.tensor_scalar(
                svecR[:], iota_pf[0:64, :], boxr64[0:64, :], cs0r0[0:64, :], OP.mult, OP.add
            )
            drT = workp.tile([64, 64], F32)
            nc.vector.tensor_scalar(drT[:], iota64f[0:64, :], -1.0, svecR[:], OP.mult, OP.add)
            drT2 = workp.tile([64, 64], F32)
            nc.scalar.square(drT2[:], drT[:])
            ErT = constp.tile([64, 64], BF16)
            nc.scalar.activation(ErT[:], drT2[:], AF.Exp, scale=-1.0)

            # ---- col exp table EcT[j, c] = exp(-(src_c[j] - c)^2), bf16
            boxc64 = constp.tile([128, 1], F32)
            nc.vector.scalar_tensor_tensor(
                boxc64[:], cs[:, 3:4], cs[:, 2:3], c64[:], OP.subtract, OP.mult
            )
            svecC = workp.tile([64, 1], F32)
            nc.vector.tensor_scalar(
                svecC[:], iota_pf[0:64, :], boxc64[0:64, :], cs[0:64, 2:3], OP.mult, OP.add
            )
            dcT = workp.tile([64, 256], F32)
            nc.vector.tensor_scalar(dcT[:], iota256f[:], -1.0, svecC[:], OP.mult, OP.add)
            dcT2 = workp.tile([64, 256], F32)
            nc.vector.tensor_tensor(dcT2[:], dcT[:], dcT[:], OP.mult)
            EcT = constp.tile([64, 256], BF16)
            nc.scalar.activation(EcT[:], dcT2[:], AF.Exp, scale=-1.0)

            # ---- normalizers via PE ones-matmuls, shared PSUM bank
            zall_ps = ps_z.tile([1, 320], F32, tag="z")
            nc.tensor.matmul(zall_ps[0:1, 0:64], ones64_bf[:], ErT[:])
            nc.tensor.matmul(zall_ps[0:1, 64:320], ones64_bf[:], EcT[:])
            lnz = workp.tile([1, 320], F32)
            nc.scalar.activation(lnz[:], zall_ps[0:1, :], AF.Ln, bias=eps1[:])
            rzall = workp.tile([1, 320], F32)
            nc.scalar.activation(rzall[:], lnz[:], AF.Exp, scale=-1.0)
            rzr = rzall[0:1, 0:64]
            rzc = rzall[0:1, 64:320]

            # ---- valid flag (only the non-tautological conditions;
            # inputs are sorted in [0,1] so 0<=x1<=x2<=256 always holds)
            v4 = workp.tile([128, 1], F32)
            nc.vector.tensor_tensor(v4[:], cs[:, 1:2], cs[:, 0:1], OP.is_gt)
            v5 = workp.tile([128, 1], F32)
            nc.vector.tensor_tensor(v5[:], cs[:, 3:4], cs[:, 2:3], OP.is_gt)
            valid = constp.tile([128, 1], F32)
            nc.vector.tensor_tensor(valid[:], v4[:], v5[:], OP.mult)

            # ---- box-interval rows/cols (gpsimd) for the mask outers
            cs1r0 = workp.tile([128, 1], F32)
            nc.vector.tensor_tensor(cs1r0[:], cs[:, 1:2], xb[:, 19:20], OP.subtract)
            r_ge = workp.tile([1, 64], F32)
            nc.vector.tensor_scalar(r_ge[:], iota64f[0:1, :], cs0r0[0:1, :], None, OP.is_ge)
            r_lt = workp.tile([1, 64], F32)
            nc.vector.tensor_scalar(r_lt[:], iota64f[0:1, :], cs1r0[0:1, :], None, OP.is_lt)
            rowv2 = workp.tile([1, 128], BF16)  # valid * inside_r, stacked x2
            nc.vector.scalar_tensor_tensor(
                rowv2[0:1, 0:64], r_ge[:], valid[0:1, :], r_lt[:], OP.mult, OP.mult
            )
            nc.vector.tensor_copy(rowv2[0:1, 64:128], rowv2[0:1, 0:64])
            c_ge = workp.tile([1, 256], F32)
            nc.vector.tensor_scalar(c_ge[:], iota256f[0:1, :], cs[0:1, 2:3], None, OP.is_ge)
            c_lt = workp.tile([1, 256], F32)
            nc.vector.tensor_scalar(c_lt[:], iota256f[0:1, :], cs[0:1, 3:4], None, OP.is_lt)
            colin = workp.tile([1, 256], BF16)
            nc.vector.tensor_tensor(colin[:], c_ge[:], c_lt[:], OP.mult)

            # ---- scaled mask factor rows (bf16 for 1-pass PE outers)
            rowz2 = workp.tile([1, 128], BF16)
            nc.vector.tensor_tensor(rowz2[0:1, 0:64], rowv2[0:1, 0:64], rzr, OP.mult)
            nc.vector.tensor_copy(rowz2[0:1, 64:128], rowz2[0:1, 0:64])
            colz = workp.tile([1, 256], BF16)
            nc.vector.tensor_tensor(colz[:], colin[:], rzc, OP.mult)

            # ---- mask outer products on PE
            mz_ps = ps_m.tile([128, 256], F32, tag="m", name="mzps")
            nc.tensor.matmul(mz_ps[:], rowz2[:], colz[:])
            mv_ps = ps_m.tile([128, 256], F32, tag="m", name="mvps")
            nc.tensor.matmul(mv_ps[:], rowv2[:], colin[:])
            mz_sb = constp.tile([128, 256], F32)
            nc.scalar.copy(mz_sb[:], mz_ps[:])
            w2 = constp.tile([128, 256], F32)
            nc.vector.tensor_scalar(w2[:], mv_ps[:], -1.0, valid[:], OP.mult, OP.add)

            # ---- t1[ch][j, r] = sum_i wimg[i, (ch,j)] * ErT[i, r]
            t1_ps = ps_t1.tile([64, 192], F32, tag="t1")
            for ch in range(3):
                nc.tensor.matmul(
                    t1_ps[:, 64 * ch : 64 * (ch + 1)],
                    wimg[:, 64 * ch : 64 * (ch + 1)],
                    ErT[:],
                )
            t1all = constp.tile([64, 192], BF16)
            nc.scalar.copy(t1all[:, 0:64], t1_ps[:, 0:64])
            nc.vector.tensor_copy(t1all[:, 64:128], t1_ps[:, 64:128])
            nc.vector.tensor_copy(t1all[:, 128:192], t1_ps[:, 128:192])

            # ---- eR matmuls
            er_ab_ps = ps_r.tile([128, 256], F32, tag="rab", name="erab")
            nc.tensor.matmul(er_ab_ps[:], t1all[:, 0:128], EcT[:])
            er_c_ps = ps_r.tile([64, 256], F32, tag="rc", name="erc")
            nc.tensor.matmul(er_c_ps[:], t1all[:, 128:192], EcT[:])

            # ---- blend: res = eR*Mz + (valid - Mv)
            res_ab = outp.tile([128, 256], F32)
            nc.vector.tensor_tensor(res_ab[:], er_ab_ps[:], mz_sb[:], OP.mult)
            nc.vector.tensor_tensor(res_ab[:], res_ab[:], w2[:], OP.add)
            res_c = outp.tile([64, 256], F32)
            nc.vector.tensor_tensor(res_c[:], er_c_ps[:], mz_sb[0:64, :], OP.mult)
            nc.vector.tensor_tensor(res_c[:], res_c[:], w2[0:64, :], OP.add)

            # ---- output DMAs on the two HWDGE rings in parallel
            nc.sync.dma_start(
                out_d[0:2, :, :].rearrange("a b c -> (a b) c"), res_ab[:]
            )
            nc.scalar.dma_start(out_d[2, :, :], res_c[:])

    nc.compile()
    return nc


_CACHE = {}


def get_nc():
    if "nc" not in _CACHE:
        _CACHE["nc"] = build_nc()
    return _CACHE["nc"]


def make_in_maps(X, images):
    X = np.ascontiguousarray(np.asarray(X, np.float32))
    images = np.ascontiguousarray(np.asarray(images, np.float32))
    # layout/dtype prep only: [14,4,64,64] f32 -> [14, 64(i), 3*64(ch,j)] bf16
    imgs_gt = np.ascontiguousarray(
        images[:, 0:3].transpose(0, 2, 1, 3).reshape(N_IMG, S, 3 * S)
    ).astype(ml_dtypes.bfloat16)
    in_maps = []
    for c in range(N_CORES):
        pic, rb = divmod(c, 4)
        xm = np.zeros((1, 24), np.float32)
        xm[0, :19] = X[pic, 0]
        xm[0, 19] = float(RB * rb)
        in_maps.append({"xmeta": np.tile(xm, (128, 1)), "imgs": imgs_gt})
    return in_maps


def assemble(results):
    out = np.empty((2, 3, H, H), np.float32)
    for c in range(N_CORES):
        pic, rb = divmod(c, 4)
        out[pic, :, RB * rb : RB * (rb + 1), :] = results[c]["out"]
    return out


def _axon_reset():
    try:
        import ctypes

        import jax

        jax.devices()
        ctypes.CDLL("/opt/axon/libaxon_pjrt.so").axon_reset()
    except Exception:
        pass


def kernel(X, images):
    nc = get_nc()
    in_maps = make_in_maps(X, images)
    try:
        res = run_bass_kernel_spmd(nc, in_maps, list(range(N_CORES)))
    except Exception:
        # the axon terminal can be left in a bad state by earlier failed
        # runs (LoadExecutable errors); reset and retry once
        _axon_reset()
        res = run_bass_kernel_spmd(nc, in_maps, list(range(N_CORES)))
    return assemble(res.results)


# revision 17
# speedup vs baseline: 1.2912x; 1.0780x over previous
"""Trainium2 Bass kernel for the emoji-box decoder problem (optimized v2).

Math: per picture, softmax(-d2) over emoji pixels is separable:
softmax_r (x) softmax_c.  This version postpones BOTH softmax
normalizations: it computes unnormalized ErT[i,r] = exp(-(src_r[i]-r)^2)
and EcT[j,c] = exp(-(src_c[j]-c)^2) DIRECTLY in transposed layout
(partition = emoji index), so no PE transposes are needed.  The
row normalizer Zr = sum_i ErT comes from a PE ones-matmul and 1/Zr is
folded into the box mask outer product.  The column softmax Ac is
computed in natural [c, j] layout (fast per-partition normalize) and
PE-transposed to AcT:
    res = eR * Mz + (valid - Mv)
    eR  = t1[j,(ch,r)] = sum_i wimg[i,(ch,j)] ErT[i,r],
          eR[(ch,r),c] = sum_j t1 AcT[j,c]
    Mz  = (valid*rowin*rzr) (x) colin,  Mv = (valid*rowin) (x) colin
which equals where(valid, where(inside, R, 1), 0).

Emoji selection: argmax of logits -> gpsimd register -> dynamic-offset
DRAM->SBUF DMA gather of the selected emoji (bf16, i-major layout so
each partition reads one contiguous 384B chunk).

Matmuls run in bf16 (1-pass PE) — images are cast to bf16 on the host
(layout/dtype prep only), exp outputs are written as bf16.

Sharding: 8 cores = 2 pictures x 4 row-blocks of 64 canvas rows.
xmeta is host-replicated to all 128 partitions (no on-device broadcast).
"""

import sys

import numpy as np

if "/opt/trn_rl_repo" not in sys.path:
    sys.path.insert(0, "/opt/trn_rl_repo")

import ml_dtypes

import concourse.bacc as bacc
import concourse.bass as bass
import concourse.mybir as mybir
import concourse.tile as tile
from concourse.bass_utils import run_bass_kernel_spmd


def _ensure_ntff_hook():
    """The image's antenv package lacks axon_hooks, so trn_boot's NTFF
    profile hook install degrades silently and run_bass_kernel_spmd
    crashes on `from antenv.axon_hooks import ...` when trace=True.
    Provide the module and install the ctypes hook ourselves."""
    import types

    try:
        from antenv.axon_hooks import get_axon_ntff_profile_hook  # noqa: F401

        return
    except ImportError:
        pass
    mod = types.ModuleType("antenv.axon_hooks")
    _hook = [None]
    mod.set_axon_ntff_profile_hook = lambda h: _hook.__setitem__(0, h)
    mod.get_axon_ntff_profile_hook = lambda: _hook[0]
    try:
        import antenv

        sys.modules["antenv.axon_hooks"] = mod
        antenv.axon_hooks = mod
        from trn_agent_boot.trn_boot import _ntff_profile_via_ctypes

        hook = _ntff_profile_via_ctypes("/opt/axon/libaxon_pjrt.so")
        if hook is not None:
            mod.set_axon_ntff_profile_hook(hook)
    except Exception:
        pass


_ensure_ntff_hook()

F32 = mybir.dt.float32
BF16 = mybir.dt.bfloat16
I32 = mybir.dt.int32
AF = mybir.ActivationFunctionType
OP = mybir.AluOpType
AX = mybir.AxisListType

MAGIC = 8388608.0  # 2**23; x + MAGIC - MAGIC == rint(x) for 0 <= x < 2**22

N_CORES = 8
H = 256
S = 64
N_IMG = 14
RB = 64  # canvas rows per core


def build_nc():
    nc = bacc.Bacc("TRN2", target_bir_lowering=False, debug=False)

    xmeta_d = nc.dram_tensor("xmeta", [128, 24], F32, kind="ExternalInput")
    imgs_d = nc.dram_tensor("imgs", [N_IMG, S, 3 * S], BF16, kind="ExternalInput")
    out_d = nc.dram_tensor("out", [3, RB, H], F32, kind="ExternalOutput")

    with tile.TileContext(nc) as tc:
        with (
            tc.tile_pool(name="constp", bufs=1) as constp,
            tc.tile_pool(name="workp", bufs=2) as workp,
            tc.tile_pool(name="outp", bufs=1) as outp,
            tc.tile_pool(name="ps_z", bufs=1, space="PSUM") as ps_z,
            tc.tile_pool(name="ps_m", bufs=1, space="PSUM") as ps_m,
            tc.tile_pool(name="ps_t1", bufs=1, space="PSUM") as ps_t1,
            tc.tile_pool(name="ps_r", bufs=1, space="PSUM") as ps_r,
        ):
            # ---- warm the scalar-engine activation table early so the
            # ~1.3us ACT_TABLE_LOAD overlaps the input DMA
            warm = workp.tile([1, 1], F32)
            nc.gpsimd.memset(warm[:], 0.0)
            warm2 = workp.tile([1, 1], F32)
            nc.scalar.activation(warm2[:], warm[:], AF.Exp)

            # ---- input DMAs: xmeta (sync HWDGE) + all-images prefetch
            # (scalar HWDGE, separate ring) issued back-to-back at entry
            xb = constp.tile([128, 24], F32)
            nc.sync.dma_start(xb[:], xmeta_d[:])

            # ---- constants / iotas (gpsimd+vector, overlap the DMAs)
            iota_pi = constp.tile([128, 1], I32)
            nc.gpsimd.iota(iota_pi[:], pattern=[[1, 1]], base=0, channel_multiplier=1)
            iota_pf = constp.tile([128, 1], F32)
            nc.vector.tensor_copy(iota_pf[:], iota_pi[:])
            iota14i = constp.tile([1, N_IMG], I32)
            nc.gpsimd.iota(iota14i[:], pattern=[[1, N_IMG]], base=0, channel_multiplier=0)
            iota14f = constp.tile([1, N_IMG], F32)
            nc.vector.tensor_copy(iota14f[:], iota14i[:])
            iota64i = constp.tile([128, 64], I32)
            nc.gpsimd.iota(iota64i[:], pattern=[[1, 64]], base=0, channel_multiplier=0)
            iota64f = constp.tile([128, 64], F32)
            nc.vector.tensor_copy(iota64f[:], iota64i[:])
            iota256i = constp.tile([64, 256], I32)
            nc.gpsimd.iota(iota256i[:], pattern=[[1, 256]], base=0, channel_multiplier=0)
            iota256f = constp.tile([64, 256], F32)
            nc.vector.tensor_copy(iota256f[:], iota256i[:])
            c64 = constp.tile([128, 1], F32)
            nc.vector.memset(c64[:], 1.0 / 64.0)
            ones64_bf = constp.tile([64, 1], BF16)
            nc.gpsimd.memset(ones64_bf[:], 1.0)
            eps1 = constp.tile([1, 1], F32)
            nc.gpsimd.memset(eps1[:], 1e-30)
            onebf = workp.tile([128, 128], BF16)
            nc.gpsimd.memset(onebf[:], 1.0)
            idbf = constp.tile([128, 128], BF16)
            nc.gpsimd.affine_select(
                idbf[:],
                onebf[:],
                pattern=[[1, 128]],
                compare_op=OP.is_equal,
                fill=0.0,
                base=0,
                channel_multiplier=-1,
            )

            # ================= after xmeta arrives =================
            # ---- emoji index chain (gpsimd) -> sync register -> gather
            rmax = workp.tile([1, 1], F32)
            nc.vector.tensor_reduce(rmax[:], xb[0:1, 5:19], AX.X, OP.max)
            dotj = workp.tile([1, N_IMG], I32)
            nc.vector.scalar_tensor_tensor(
                dotj[:], xb[0:1, 5:19], rmax[:], iota14f[:], OP.is_ge, OP.mult
            )
            idxi = workp.tile([1, 1], I32)
            with nc.allow_low_precision(reason="argmax index sum is exact in i32"):
                nc.vector.tensor_reduce(idxi[:], dotj[:], AX.X, OP.add)
            wimg = constp.tile([S, 3 * S], BF16)
            with nc.gpsimd.register("ridx") as ridx:
                nc.gpsimd.reg_load(ridx, idxi[0:1, 0:1])
                off = nc.gpsimd.snap(ridx)
                nc.gpsimd.dma_start(
                    wimg[:], imgs_d[bass.ds(off, 1), :, :].squeeze(0)
                )

            # ---- rounded box coords cs = rint(256 * X[0:4]) (vector)
            cs = constp.tile([128, 4], F32)
            nc.vector.tensor_scalar(cs[:], xb[:, 0:4], 256.0, MAGIC, OP.mult, OP.add)
            nc.vector.tensor_scalar(cs[:], cs[:], MAGIC, None, OP.subtract)

            # ---- row exp table ErT[i, r] = exp(-(src_r[i] - (r0+r))^2), bf16
            boxr64 = constp.tile([128, 1], F32)
            nc.vector.scalar_tensor_tensor(
                boxr64[:], cs[:, 1:2], cs[:, 0:1], c64[:], OP.subtract, OP.mult
            )
            cs0r0 = workp.tile([128, 1], F32)
            nc.vector.tensor_tensor(cs0r0[:], cs[:, 0:1], xb[:, 19:20], OP.subtract)
            svecR = workp.tile([64, 1], F32)
            nc.vector.tensor_scalar(
                svecR[:], iota_pf[0:64, :], boxr64[0:64, :], cs0r0[0:64, :], OP.mult, OP.add
            )
            drT = workp.tile([64, 64], F32)
            nc.vector.tensor_scalar(drT[:], iota64f[0:64, :], -1.0, svecR[:], OP.mult, OP.add)
            drT2 = workp.tile([64, 64], F32)
            nc.scalar.square(drT2[:], drT[:])
            ErT = constp.tile([64, 64], BF16)
            nc.scalar.activation(ErT[:], drT2[:], AF.Exp, scale=-1.0)

            # ---- col exp table EcT[j, c] = exp(-(src_c[j] - c)^2), bf16
            boxc64 = constp.tile([128, 1], F32)
            nc.vector.scalar_tensor_tensor(
                boxc64[:], cs[:, 3:4], cs[:, 2:3], c64[:], OP.subtract, OP.mult
            )
            # src_c[j] grid as a row (same for every canvas column partition)
            srcc = workp.tile([128, 64], F32)
            nc.vector.tensor_scalar(
                srcc[:], iota64f[:], boxc64[:], cs[:, 2:3], OP.mult, OP.add
            )
            AcT = constp.tile([64, 256], BF16)
            for t in range(2):
                c_idx = workp.tile([128, 1], F32, tag="c_idx")
                nc.vector.tensor_scalar(
                    c_idx[:], iota_pf[:], float(128 * t), None, OP.add
                )
                dc = workp.tile([128, 64], F32, tag="dc")
                nc.vector.tensor_scalar(dc[:], srcc[:], c_idx[:], None, OP.subtract)
                dc2 = workp.tile([128, 64], F32, tag="dc2")
                nc.scalar.square(dc2[:], dc[:])
                ec = workp.tile([128, 64], F32, tag="ec")
                zc = workp.tile([128, 1], F32, tag="zc")
                nc.scalar.activation(
                    ec[:], dc2[:], AF.Exp, scale=-1.0, accum_out=zc[:]
                )
                rzc_t = workp.tile([128, 1], F32, tag="rzc")
                nc.vector.tensor_scalar(rzc_t[:], zc[:], 1e-30, None, OP.add)
                nc.vector.reciprocal(rzc_t[:], rzc_t[:])
                Ac = workp.tile([128, 64], BF16, tag="Ac")
                nc.vector.tensor_scalar(Ac[:], ec[:], rzc_t[:], None, OP.mult)
                acT_ps = ps_z.tile([64, 128], BF16, tag=f"acT{t}", name=f"acTps{t}")
                nc.tensor.transpose(acT_ps[:], Ac[:], idbf[:])
                nc.vector.tensor_copy(AcT[:, 128 * t : 128 * (t + 1)], acT_ps[:])

            # ---- row normalizer Zr via PE ones-matmul (row layout)
            zr_ps = ps_z.tile([1, 64], F32, tag="z")
            nc.tensor.matmul(zr_ps[:], ones64_bf[:], ErT[:])
            rzr_row = workp.tile([1, 64], F32)
            nc.vector.tensor_scalar(rzr_row[:], zr_ps[:], 1e-30, None, OP.add)
            nc.vector.reciprocal(rzr_row[:], rzr_row[:])

            # ---- valid flag (only the non-tautological conditions;
            # inputs are sorted in [0,1] so 0<=x1<=x2<=256 always holds)
            v4 = workp.tile([128, 1], F32)
            nc.vector.tensor_tensor(v4[:], cs[:, 1:2], cs[:, 0:1], OP.is_gt)
            v5 = workp.tile([128, 1], F32)
            nc.vector.tensor_tensor(v5[:], cs[:, 3:4], cs[:, 2:3], OP.is_gt)
            valid = constp.tile([128, 1], F32)
            nc.vector.tensor_tensor(valid[:], v4[:], v5[:], OP.mult)

            # ---- box-interval rows/cols (gpsimd) for the mask outers
            cs1r0 = workp.tile([128, 1], F32)
            nc.vector.tensor_tensor(cs1r0[:], cs[:, 1:2], xb[:, 19:20], OP.subtract)
            r_ge = workp.tile([1, 64], F32)
            nc.vector.tensor_scalar(r_ge[:], iota64f[0:1, :], cs0r0[0:1, :], None, OP.is_ge)
            r_lt = workp.tile([1, 64], F32)
            nc.vector.tensor_scalar(r_lt[:], iota64f[0:1, :], cs1r0[0:1, :], None, OP.is_lt)
            rowv2 = workp.tile([1, 128], BF16)  # valid * inside_r, stacked x2
            nc.vector.scalar_tensor_tensor(
                rowv2[0:1, 0:64], r_ge[:], valid[0:1, :], r_lt[:], OP.mult, OP.mult
            )
            nc.vector.tensor_copy(rowv2[0:1, 64:128], rowv2[0:1, 0:64])
            c_ge = workp.tile([1, 256], F32)
            nc.vector.tensor_scalar(c_ge[:], iota256f[0:1, :], cs[0:1, 2:3], None, OP.is_ge)
            c_lt = workp.tile([1, 256], F32)
            nc.vector.tensor_scalar(c_lt[:], iota256f[0:1, :], cs[0:1, 3:4], None, OP.is_lt)
            colin = workp.tile([1, 256], BF16)
            nc.vector.tensor_tensor(colin[:], c_ge[:], c_lt[:], OP.mult)

            # ---- scaled mask factor rows (bf16 for 1-pass PE outers)
            rowz2 = workp.tile([1, 128], BF16)
            nc.vector.tensor_tensor(rowz2[0:1, 0:64], rowv2[0:1, 0:64], rzr_row[:], OP.mult)
            nc.vector.tensor_copy(rowz2[0:1, 64:128], rowz2[0:1, 0:64])

            # ---- mask outer products on PE
            mz_ps = ps_m.tile([128, 256], F32, tag="mz", name="mzps")
            nc.tensor.matmul(mz_ps[:], rowz2[:], colin[:])
            mv_ps = ps_m.tile([128, 256], F32, tag="mv", name="mvps")
            nc.tensor.matmul(mv_ps[:], rowv2[:], colin[:])
            mz_sb = constp.tile([128, 256], F32)
            nc.scalar.copy(mz_sb[:], mz_ps[:])
            w2 = constp.tile([128, 256], F32)
            nc.vector.tensor_scalar(w2[:], mv_ps[:], -1.0, valid[:], OP.mult, OP.add)

            # ---- t1[ch][j, r] = sum_i wimg[i, (ch,j)] * ErT[i, r]
            t1_ps = ps_t1.tile([64, 192], F32, tag="t1")
            for ch in range(3):
                nc.tensor.matmul(
                    t1_ps[:, 64 * ch : 64 * (ch + 1)],
                    wimg[:, 64 * ch : 64 * (ch + 1)],
                    ErT[:],
                )
            t1all = constp.tile([64, 192], BF16)
            nc.scalar.copy(t1all[:, 0:64], t1_ps[:, 0:64])
            nc.vector.tensor_copy(t1all[:, 64:128], t1_ps[:, 64:128])
            nc.vector.tensor_copy(t1all[:, 128:192], t1_ps[:, 128:192])

            # ---- eR matmuls
            er_ab_ps = ps_r.tile([128, 256], F32, tag="rab", name="erab")
            nc.tensor.matmul(er_ab_ps[:], t1all[:, 0:128], AcT[:])
            er_c_ps = ps_r.tile([64, 256], F32, tag="rc", name="erc")
            nc.tensor.matmul(er_c_ps[:], t1all[:, 128:192], AcT[:])

            # ---- blend: res = eR*Mz + (valid - Mv)
            res_ab = outp.tile([128, 256], F32)
            nc.vector.tensor_tensor(res_ab[:], er_ab_ps[:], mz_sb[:], OP.mult)
            nc.vector.tensor_tensor(res_ab[:], res_ab[:], w2[:], OP.add)
            res_c = outp.tile([64, 256], F32)
            nc.vector.tensor_tensor(res_c[:], er_c_ps[:], mz_sb[0:64, :], OP.mult)
            nc.vector.tensor_tensor(res_c[:], res_c[:], w2[0:64, :], OP.add)

            # ---- output DMAs on the two HWDGE rings in parallel
            nc.sync.dma_start(
                out_d[0:2, :, :].rearrange("a b c -> (a b) c"), res_ab[:]
            )
            nc.scalar.dma_start(out_d[2, :, :], res_c[:])

    nc.compile()
    return nc


_CACHE = {}


def get_nc():
    if "nc" not in _CACHE:
        _CACHE["nc"] = build_nc()
    return _CACHE["nc"]


def make_in_maps(X, images):
    X = np.ascontiguousarray(np.asarray(X, np.float32))
    images = np.ascontiguousarray(np.asarray(images, np.float32))
    # layout/dtype prep only: [14,4,64,64] f32 -> [14, 64(i), 3*64(ch,j)] bf16
    imgs_gt = np.ascontiguousarray(
        images[:, 0:3].transpose(0, 2, 1, 3).reshape(N_IMG, S, 3 * S)
    ).astype(ml_dtypes.bfloat16)
    in_maps = []
    for c in range(N_CORES):
        pic, rb = divmod(c, 4)
        xm = np.zeros((1, 24), np.float32)
        xm[0, :19] = X[pic, 0]
        xm[0, 19] = float(RB * rb)
        in_maps.append({"xmeta": np.tile(xm, (128, 1)), "imgs": imgs_gt})
    return in_maps


def assemble(results):
    out = np.empty((2, 3, H, H), np.float32)
    for c in range(N_CORES):
        pic, rb = divmod(c, 4)
        out[pic, :, RB * rb : RB * (rb + 1), :] = results[c]["out"]
    return out


def _axon_reset():
    try:
        import ctypes

        import jax

        jax.devices()
        ctypes.CDLL("/opt/axon/libaxon_pjrt.so").axon_reset()
    except Exception:
        pass


def kernel(X, images):
    nc = get_nc()
    in_maps = make_in_maps(X, images)
    try:
        res = run_bass_kernel_spmd(nc, in_maps, list(range(N_CORES)))
    except Exception:
        # the axon terminal can be left in a bad state by earlier failed
        # runs (LoadExecutable errors); reset and retry once
        _axon_reset()
        res = run_bass_kernel_spmd(nc, in_maps, list(range(N_CORES)))
    return assemble(res.results)


# revision 19
# speedup vs baseline: 1.3230x; 1.0246x over previous
"""Trainium2 Bass kernel for the emoji-box decoder problem (optimized v2).

Math: per picture, softmax(-d2) over emoji pixels is separable:
softmax_r (x) softmax_c.  This version postpones BOTH softmax
normalizations: it computes unnormalized ErT[i,r] = exp(-(src_r[i]-r)^2)
and EcT[j,c] = exp(-(src_c[j]-c)^2) DIRECTLY in transposed layout
(partition = emoji index), so no PE transposes are needed.  The
row normalizer Zr = sum_i ErT comes from a PE ones-matmul and 1/Zr is
folded into the box mask outer product.  The column softmax Ac is
computed in natural [c, j] layout (fast per-partition normalize) and
PE-transposed to AcT:
    res = eR * Mz + (valid - Mv)
    eR  = t1[j,(ch,r)] = sum_i wimg[i,(ch,j)] ErT[i,r],
          eR[(ch,r),c] = sum_j t1 AcT[j,c]
    Mz  = (valid*rowin*rzr) (x) colin,  Mv = (valid*rowin) (x) colin
which equals where(valid, where(inside, R, 1), 0).

Emoji selection: argmax of logits -> gpsimd register -> dynamic-offset
DRAM->SBUF DMA gather of the selected emoji (bf16, i-major layout so
each partition reads one contiguous 384B chunk).

Matmuls run in bf16 (1-pass PE) — images are cast to bf16 on the host
(layout/dtype prep only), exp outputs are written as bf16.

Sharding: 8 cores = 2 pictures x 4 row-blocks of 64 canvas rows.
xmeta is host-replicated to all 128 partitions (no on-device broadcast).
"""

import sys

import numpy as np

if "/opt/trn_rl_repo" not in sys.path:
    sys.path.insert(0, "/opt/trn_rl_repo")

import ml_dtypes

import concourse.bacc as bacc
import concourse.bass as bass
import concourse.mybir as mybir
import concourse.tile as tile
from concourse.bass_utils import run_bass_kernel_spmd


def _ensure_ntff_hook():
    """The image's antenv package lacks axon_hooks, so trn_boot's NTFF
    profile hook install degrades silently and run_bass_kernel_spmd
    crashes on `from antenv.axon_hooks import ...` when trace=True.
    Provide the module and install the ctypes hook ourselves."""
    import types

    try:
        from antenv.axon_hooks import get_axon_ntff_profile_hook  # noqa: F401

        return
    except ImportError:
        pass
    mod = types.ModuleType("antenv.axon_hooks")
    _hook = [None]
    mod.set_axon_ntff_profile_hook = lambda h: _hook.__setitem__(0, h)
    mod.get_axon_ntff_profile_hook = lambda: _hook[0]
    try:
        import antenv

        sys.modules["antenv.axon_hooks"] = mod
        antenv.axon_hooks = mod
        from trn_agent_boot.trn_boot import _ntff_profile_via_ctypes

        hook = _ntff_profile_via_ctypes("/opt/axon/libaxon_pjrt.so")
        if hook is not None:
            mod.set_axon_ntff_profile_hook(hook)
    except Exception:
        pass


_ensure_ntff_hook()

F32 = mybir.dt.float32
BF16 = mybir.dt.bfloat16
I32 = mybir.dt.int32
AF = mybir.ActivationFunctionType
OP = mybir.AluOpType
AX = mybir.AxisListType

MAGIC = 8388608.0  # 2**23; x + MAGIC - MAGIC == rint(x) for 0 <= x < 2**22

N_CORES = 8
H = 256
S = 64
N_IMG = 14
RB = 64  # canvas rows per core


def build_nc():
    nc = bacc.Bacc("TRN2", target_bir_lowering=False, debug=False)

    xmeta_d = nc.dram_tensor("xmeta", [128, 24], F32, kind="ExternalInput")
    xrow_d = nc.dram_tensor("xrow", [1, 24], F32, kind="ExternalInput")
    imgs_d = nc.dram_tensor("imgs", [N_IMG, S, 3 * S], BF16, kind="ExternalInput")
    out_d = nc.dram_tensor("out", [3, RB, H], F32, kind="ExternalOutput")

    with tile.TileContext(nc) as tc:
        with (
            tc.tile_pool(name="constp", bufs=1) as constp,
            tc.tile_pool(name="workp", bufs=2) as workp,
            tc.tile_pool(name="outp", bufs=1) as outp,
            tc.tile_pool(name="ps_z", bufs=1, space="PSUM") as ps_z,
            tc.tile_pool(name="ps_m", bufs=1, space="PSUM") as ps_m,
            tc.tile_pool(name="ps_t1", bufs=1, space="PSUM") as ps_t1,
            tc.tile_pool(name="ps_r", bufs=1, space="PSUM") as ps_r,
        ):
            # ---- input DMAs first: xrow (sync ring, 1 descriptor -> fast
            # round trip, feeds the argmax/gather chain) and the broadcast
            # copy xb (scalar ring, emitted before the warm activation so
            # the DMA issue precedes the ACT_TABLE_LOAD on that queue)
            xrow = constp.tile([1, 24], F32)
            nc.sync.dma_start(xrow[:], xrow_d[:])
            xb = constp.tile([128, 24], F32)
            nc.scalar.dma_start(xb[:], xmeta_d[:])

            # ---- warm the scalar-engine activation table early so the
            # ~1.3us ACT_TABLE_LOAD overlaps the input DMAs
            warm = workp.tile([1, 1], F32)
            nc.gpsimd.memset(warm[:], 0.0)
            warm2 = workp.tile([1, 1], F32)
            nc.scalar.activation(warm2[:], warm[:], AF.Exp)

            # ---- constants / iotas (gpsimd+vector, overlap the DMAs)
            iota_pi = constp.tile([128, 1], I32)
            nc.gpsimd.iota(iota_pi[:], pattern=[[1, 1]], base=0, channel_multiplier=1)
            iota_pf = constp.tile([128, 1], F32)
            nc.vector.tensor_copy(iota_pf[:], iota_pi[:])
            iota14i = constp.tile([1, N_IMG], I32)
            nc.gpsimd.iota(iota14i[:], pattern=[[1, N_IMG]], base=0, channel_multiplier=0)
            iota14f = constp.tile([1, N_IMG], F32)
            nc.vector.tensor_copy(iota14f[:], iota14i[:])
            iota64i = constp.tile([128, 64], I32)
            nc.gpsimd.iota(iota64i[:], pattern=[[1, 64]], base=0, channel_multiplier=0)
            iota64f = constp.tile([128, 64], F32)
            nc.vector.tensor_copy(iota64f[:], iota64i[:])
            iota256i = constp.tile([64, 256], I32)
            nc.gpsimd.iota(iota256i[:], pattern=[[1, 256]], base=0, channel_multiplier=0)
            iota256f = constp.tile([64, 256], F32)
            nc.vector.tensor_copy(iota256f[:], iota256i[:])
            c64 = constp.tile([128, 1], F32)
            nc.vector.memset(c64[:], 1.0 / 64.0)
            ones64_bf = constp.tile([64, 1], BF16)
            nc.gpsimd.memset(ones64_bf[:], 1.0)
            eps1 = constp.tile([1, 1], F32)
            nc.gpsimd.memset(eps1[:], 1e-30)
            onebf = workp.tile([128, 128], BF16)
            nc.gpsimd.memset(onebf[:], 1.0)
            idbf = constp.tile([128, 128], BF16)
            nc.gpsimd.affine_select(
                idbf[:],
                onebf[:],
                pattern=[[1, 128]],
                compare_op=OP.is_equal,
                fill=0.0,
                base=0,
                channel_multiplier=-1,
            )

            # ================= after xmeta arrives =================
            # ---- emoji index chain (gpsimd) -> sync register -> gather
            rmax = workp.tile([1, 1], F32)
            nc.vector.tensor_reduce(rmax[:], xrow[0:1, 5:19], AX.X, OP.max)
            dotj = workp.tile([1, N_IMG], I32)
            nc.vector.scalar_tensor_tensor(
                dotj[:], xrow[0:1, 5:19], rmax[:], iota14f[:], OP.is_ge, OP.mult
            )
            idxi = workp.tile([1, 1], I32)
            with nc.allow_low_precision(reason="argmax index sum is exact in i32"):
                nc.vector.tensor_reduce(idxi[:], dotj[:], AX.X, OP.add)
            wimg = constp.tile([S, 3 * S], BF16)
            with nc.sync.register("ridx") as ridx:
                nc.sync.reg_load(ridx, idxi[0:1, 0:1])
                off = nc.sync.snap(ridx)
                nc.sync.dma_start(
                    wimg[:], imgs_d[bass.ds(off, 1), :, :].squeeze(0)
                )

            # ---- rounded box coords cs = rint(256 * X[0:4]) (vector)
            cs = constp.tile([128, 4], F32)
            nc.vector.tensor_scalar(cs[:], xb[:, 0:4], 256.0, MAGIC, OP.mult, OP.add)
            nc.vector.tensor_scalar(cs[:], cs[:], MAGIC, None, OP.subtract)

            # ---- row exp table ErT[i, r] = exp(-(src_r[i] - (r0+r))^2), bf16
            boxr64 = constp.tile([128, 1], F32)
            nc.vector.scalar_tensor_tensor(
                boxr64[:], cs[:, 1:2], cs[:, 0:1], c64[:], OP.subtract, OP.mult
            )
            cs0r0 = workp.tile([128, 1], F32)
            nc.vector.tensor_tensor(cs0r0[:], cs[:, 0:1], xb[:, 19:20], OP.subtract)
            svecR = workp.tile([64, 1], F32)
            nc.vector.tensor_scalar(
                svecR[:], iota_pf[0:64, :], boxr64[0:64, :], cs0r0[0:64, :], OP.mult, OP.add
            )
            drT = workp.tile([64, 64], F32)
            nc.vector.tensor_scalar(drT[:], iota64f[0:64, :], -1.0, svecR[:], OP.mult, OP.add)
            drT2 = workp.tile([64, 64], F32)
            nc.scalar.square(drT2[:], drT[:])
            ErT = constp.tile([64, 64], BF16)
            nc.scalar.activation(ErT[:], drT2[:], AF.Exp, scale=-1.0)

            # ---- col exp table EcT[j, c] = exp(-(src_c[j] - c)^2), bf16
            boxc64 = constp.tile([128, 1], F32)
            nc.vector.scalar_tensor_tensor(
                boxc64[:], cs[:, 3:4], cs[:, 2:3], c64[:], OP.subtract, OP.mult
            )
            # src_c[j] grid as a row (same for every canvas column partition)
            srcc = workp.tile([128, 64], F32)
            nc.vector.tensor_scalar(
                srcc[:], iota64f[:], boxc64[:], cs[:, 2:3], OP.mult, OP.add
            )
            dc_all = workp.tile([128, 2, 64], F32)
            nc.vector.tensor_scalar(
                dc_all[:, 0, :], srcc[:], iota_pf[:], None, OP.subtract
            )
            nc.vector.tensor_scalar(
                dc_all[:, 1, :], dc_all[:, 0, :], 128.0, None, OP.subtract
            )
            dc2_all = workp.tile([128, 2, 64], F32)
            nc.scalar.square(dc2_all[:], dc_all[:])
            AcT = constp.tile([64, 256], BF16)
            for t in range(2):
                ec = workp.tile([128, 64], F32, tag="ec")
                zc = workp.tile([128, 1], F32, tag="zc")
                nc.scalar.activation(
                    ec[:], dc2_all[:, t, :], AF.Exp, scale=-1.0, accum_out=zc[:]
                )
                rzc_t = workp.tile([128, 1], F32, tag="rzc")
                nc.vector.tensor_scalar(rzc_t[:], zc[:], 1e-30, None, OP.add)
                nc.vector.reciprocal(rzc_t[:], rzc_t[:])
                Ac = workp.tile([128, 64], BF16, tag="Ac")
                nc.vector.tensor_scalar(Ac[:], ec[:], rzc_t[:], None, OP.mult)
                acT_ps = ps_z.tile([64, 128], BF16, tag=f"acT{t}", name=f"acTps{t}")
                nc.tensor.transpose(acT_ps[:], Ac[:], idbf[:])
                nc.vector.tensor_copy(AcT[:, 128 * t : 128 * (t + 1)], acT_ps[:])

            # ---- row normalizer Zr via PE ones-matmul (row layout)
            zr_ps = ps_z.tile([1, 64], F32, tag="z")
            nc.tensor.matmul(zr_ps[:], ones64_bf[:], ErT[:])
            rzr_row = workp.tile([1, 64], F32)
            nc.vector.tensor_scalar(rzr_row[:], zr_ps[:], 1e-30, None, OP.add)
            nc.vector.reciprocal(rzr_row[:], rzr_row[:])

            # ---- valid flag (only the non-tautological conditions;
            # inputs are sorted in [0,1] so 0<=x1<=x2<=256 always holds)
            v4 = workp.tile([128, 1], F32)
            nc.vector.tensor_tensor(v4[:], cs[:, 1:2], cs[:, 0:1], OP.is_gt)
            v5 = workp.tile([128, 1], F32)
            nc.vector.tensor_tensor(v5[:], cs[:, 3:4], cs[:, 2:3], OP.is_gt)
            valid = constp.tile([128, 1], F32)
            nc.vector.tensor_tensor(valid[:], v4[:], v5[:], OP.mult)

            # ---- box-interval rows/cols (gpsimd) for the mask outers
            cs1r0 = workp.tile([128, 1], F32)
            nc.vector.tensor_tensor(cs1r0[:], cs[:, 1:2], xb[:, 19:20], OP.subtract)
            r_ge = workp.tile([1, 64], F32)
            nc.vector.tensor_scalar(r_ge[:], iota64f[0:1, :], cs0r0[0:1, :], None, OP.is_ge)
            r_lt = workp.tile([1, 64], F32)
            nc.vector.tensor_scalar(r_lt[:], iota64f[0:1, :], cs1r0[0:1, :], None, OP.is_lt)
            rowv2 = workp.tile([1, 128], BF16)  # valid * inside_r, stacked x2
            nc.vector.scalar_tensor_tensor(
                rowv2[0:1, 0:64], r_ge[:], valid[0:1, :], r_lt[:], OP.mult, OP.mult
            )
            nc.vector.tensor_copy(rowv2[0:1, 64:128], rowv2[0:1, 0:64])
            c_ge = workp.tile([1, 256], F32)
            nc.vector.tensor_scalar(c_ge[:], iota256f[0:1, :], cs[0:1, 2:3], None, OP.is_ge)
            c_lt = workp.tile([1, 256], F32)
            nc.vector.tensor_scalar(c_lt[:], iota256f[0:1, :], cs[0:1, 3:4], None, OP.is_lt)
            colin = workp.tile([1, 256], BF16)
            nc.vector.tensor_tensor(colin[:], c_ge[:], c_lt[:], OP.mult)

            # ---- scaled mask factor rows (bf16 for 1-pass PE outers)
            rowz2 = workp.tile([1, 128], BF16)
            nc.vector.tensor_tensor(rowz2[0:1, 0:64], rowv2[0:1, 0:64], rzr_row[:], OP.mult)
            nc.vector.tensor_copy(rowz2[0:1, 64:128], rowz2[0:1, 0:64])

            # ---- mask outer products on PE
            mz_ps = ps_m.tile([128, 256], F32, tag="mz", name="mzps")
            nc.tensor.matmul(mz_ps[:], rowz2[:], colin[:])
            mv_ps = ps_m.tile([128, 256], F32, tag="mv", name="mvps")
            nc.tensor.matmul(mv_ps[:], rowv2[:], colin[:])
            mz_sb = constp.tile([128, 256], F32)
            nc.scalar.copy(mz_sb[:], mz_ps[:])
            w2 = constp.tile([128, 256], F32)
            nc.vector.tensor_scalar(w2[:], mv_ps[:], -1.0, valid[:], OP.mult, OP.add)

            # ---- t1[ch][j, r] = sum_i wimg[i, (ch,j)] * ErT[i, r]
            t1_ps = ps_t1.tile([64, 192], F32, tag="t1")
            for ch in range(3):
                nc.tensor.matmul(
                    t1_ps[:, 64 * ch : 64 * (ch + 1)],
                    wimg[:, 64 * ch : 64 * (ch + 1)],
                    ErT[:],
                )
            t1all = constp.tile([64, 192], BF16)
            nc.scalar.copy(t1all[:, 0:64], t1_ps[:, 0:64])
            nc.vector.tensor_copy(t1all[:, 64:128], t1_ps[:, 64:128])
            nc.vector.tensor_copy(t1all[:, 128:192], t1_ps[:, 128:192])

            # ---- eR matmuls
            er_ab_ps = ps_r.tile([128, 256], F32, tag="rab", name="erab")
            nc.tensor.matmul(er_ab_ps[:], t1all[:, 0:128], AcT[:])
            er_c_ps = ps_r.tile([64, 256], F32, tag="rc", name="erc")
            nc.tensor.matmul(er_c_ps[:], t1all[:, 128:192], AcT[:])

            # ---- blend: res = eR*Mz + (valid - Mv)
            res_ab = outp.tile([128, 256], F32)
            nc.vector.tensor_tensor(res_ab[:], er_ab_ps[:], mz_sb[:], OP.mult)
            nc.vector.tensor_tensor(res_ab[:], res_ab[:], w2[:], OP.add)
            res_c = outp.tile([64, 256], F32)
            nc.vector.tensor_tensor(res_c[:], er_c_ps[:], mz_sb[0:64, :], OP.mult)
            nc.vector.tensor_tensor(res_c[:], res_c[:], w2[0:64, :], OP.add)

            # ---- output DMAs on the two HWDGE rings in parallel
            nc.sync.dma_start(
                out_d[0:2, :, :].rearrange("a b c -> (a b) c"), res_ab[:]
            )
            nc.scalar.dma_start(out_d[2, :, :], res_c[:])

    nc.compile()
    return nc


_CACHE = {}


def get_nc():
    if "nc" not in _CACHE:
        _CACHE["nc"] = build_nc()
    return _CACHE["nc"]


def make_in_maps(X, images):
    X = np.ascontiguousarray(np.asarray(X, np.float32))
    images = np.ascontiguousarray(np.asarray(images, np.float32))
    # layout/dtype prep only: [14,4,64,64] f32 -> [14, 64(i), 3*64(ch,j)] bf16
    imgs_gt = np.ascontiguousarray(
        images[:, 0:3].transpose(0, 2, 1, 3).reshape(N_IMG, S, 3 * S)
    ).astype(ml_dtypes.bfloat16)
    in_maps = []
    for c in range(N_CORES):
        pic, rb = divmod(c, 4)
        xm = np.zeros((1, 24), np.float32)
        xm[0, :19] = X[pic, 0]
        xm[0, 19] = float(RB * rb)
        in_maps.append(
            {"xmeta": np.tile(xm, (128, 1)), "xrow": xm.copy(), "imgs": imgs_gt}
        )
    return in_maps


def assemble(results):
    out = np.empty((2, 3, H, H), np.float32)
    for c in range(N_CORES):
        pic, rb = divmod(c, 4)
        out[pic, :, RB * rb : RB * (rb + 1), :] = results[c]["out"]
    return out


def _axon_reset():
    try:
        import ctypes

        import jax

        jax.devices()
        ctypes.CDLL("/opt/axon/libaxon_pjrt.so").axon_reset()
    except Exception:
        pass


def kernel(X, images):
    nc = get_nc()
    in_maps = make_in_maps(X, images)
    try:
        res = run_bass_kernel_spmd(nc, in_maps, list(range(N_CORES)))
    except Exception:
        # the axon terminal can be left in a bad state by earlier failed
        # runs (LoadExecutable errors); reset and retry once
        _axon_reset()
        res = run_bass_kernel_spmd(nc, in_maps, list(range(N_CORES)))
    return assemble(res.results)


# revision 21
# speedup vs baseline: 1.3264x; 1.0025x over previous
"""Trainium2 Bass kernel for the emoji-box decoder problem (optimized v2).

Math: per picture, softmax(-d2) over emoji pixels is separable:
softmax_r (x) softmax_c.  This version postpones BOTH softmax
normalizations: it computes unnormalized ErT[i,r] = exp(-(src_r[i]-r)^2)
and EcT[j,c] = exp(-(src_c[j]-c)^2) DIRECTLY in transposed layout
(partition = emoji index), so no PE transposes are needed.  The
row normalizer Zr = sum_i ErT comes from a PE ones-matmul and 1/Zr is
folded into the box mask outer product.  The column softmax Ac is
computed in natural [c, j] layout (fast per-partition normalize) and
PE-transposed to AcT:
    res = eR * Mz + (valid - Mv)
    eR  = t1[j,(ch,r)] = sum_i wimg[i,(ch,j)] ErT[i,r],
          eR[(ch,r),c] = sum_j t1 AcT[j,c]
    Mz  = (valid*rowin*rzr) (x) colin,  Mv = (valid*rowin) (x) colin
which equals where(valid, where(inside, R, 1), 0).

Emoji selection: argmax of logits -> gpsimd register -> dynamic-offset
DRAM->SBUF DMA gather of the selected emoji (bf16, i-major layout so
each partition reads one contiguous 384B chunk).

Matmuls run in bf16 (1-pass PE) — images are cast to bf16 on the host
(layout/dtype prep only), exp outputs are written as bf16.

Sharding: 8 cores = 2 pictures x 4 row-blocks of 64 canvas rows.
xmeta is host-replicated to all 128 partitions (no on-device broadcast).
"""

import sys

import numpy as np

if "/opt/trn_rl_repo" not in sys.path:
    sys.path.insert(0, "/opt/trn_rl_repo")

import ml_dtypes

import concourse.bacc as bacc
import concourse.bass as bass
import concourse.mybir as mybir
import concourse.tile as tile
from concourse.bass_utils import run_bass_kernel_spmd


def _ensure_ntff_hook():
    """The image's antenv package lacks axon_hooks, so trn_boot's NTFF
    profile hook install degrades silently and run_bass_kernel_spmd
    crashes on `from antenv.axon_hooks import ...` when trace=True.
    Provide the module and install the ctypes hook ourselves."""
    import types

    try:
        from antenv.axon_hooks import get_axon_ntff_profile_hook  # noqa: F401

        return
    except ImportError:
        pass
    mod = types.ModuleType("antenv.axon_hooks")
    _hook = [None]
    mod.set_axon_ntff_profile_hook = lambda h: _hook.__setitem__(0, h)
    mod.get_axon_ntff_profile_hook = lambda: _hook[0]
    try:
        import antenv

        sys.modules["antenv.axon_hooks"] = mod
        antenv.axon_hooks = mod
        from trn_agent_boot.trn_boot import _ntff_profile_via_ctypes

        hook = _ntff_profile_via_ctypes("/opt/axon/libaxon_pjrt.so")
        if hook is not None:
            mod.set_axon_ntff_profile_hook(hook)
    except Exception:
        pass


_ensure_ntff_hook()

F32 = mybir.dt.float32
BF16 = mybir.dt.bfloat16
I32 = mybir.dt.int32
AF = mybir.ActivationFunctionType
OP = mybir.AluOpType
AX = mybir.AxisListType

MAGIC = 8388608.0  # 2**23; x + MAGIC - MAGIC == rint(x) for 0 <= x < 2**22

N_CORES = 8
H = 256
S = 64
N_IMG = 14
RB = 64  # canvas rows per core


def build_nc():
    nc = bacc.Bacc("TRN2", target_bir_lowering=False, debug=False)

    xmeta_d = nc.dram_tensor("xmeta", [128, 24], F32, kind="ExternalInput")
    xrow_d = nc.dram_tensor("xrow", [1, 24], F32, kind="ExternalInput")
    imgs_d = nc.dram_tensor("imgs", [N_IMG, S, 3 * S], BF16, kind="ExternalInput")
    out_d = nc.dram_tensor("out", [3, RB, H], F32, kind="ExternalOutput")

    with tile.TileContext(nc) as tc:
        with (
            tc.tile_pool(name="constp", bufs=1) as constp,
            tc.tile_pool(name="workp", bufs=2) as workp,
            tc.tile_pool(name="outp", bufs=1) as outp,
            tc.tile_pool(name="ps_z", bufs=1, space="PSUM") as ps_z,
            tc.tile_pool(name="ps_m", bufs=1, space="PSUM") as ps_m,
            tc.tile_pool(name="ps_t1", bufs=1, space="PSUM") as ps_t1,
            tc.tile_pool(name="ps_r", bufs=1, space="PSUM") as ps_r,
        ):
            # ---- input DMAs first: xrow (sync ring, 1 descriptor -> fast
            # round trip, feeds the argmax/gather chain) and the broadcast
            # copy xb (scalar ring, emitted before the warm activation so
            # the DMA issue precedes the ACT_TABLE_LOAD on that queue)
            xrow = constp.tile([1, 24], F32)
            nc.sync.dma_start(xrow[:], xrow_d[:])
            xb = constp.tile([128, 24], F32)
            nc.scalar.dma_start(xb[:], xmeta_d[:])

            # ---- warm the scalar-engine activation table early so the
            # ~1.3us ACT_TABLE_LOAD overlaps the input DMAs
            warm = workp.tile([1, 1], F32)
            nc.gpsimd.memset(warm[:], 0.0)
            warm2 = workp.tile([1, 1], F32)
            nc.scalar.activation(warm2[:], warm[:], AF.Exp)

            # ---- constants / iotas (gpsimd+vector, overlap the DMAs)
            iota_pi = constp.tile([128, 1], I32)
            nc.gpsimd.iota(iota_pi[:], pattern=[[1, 1]], base=0, channel_multiplier=1)
            iota_pf = constp.tile([128, 1], F32)
            nc.vector.tensor_copy(iota_pf[:], iota_pi[:])
            iota14i = constp.tile([1, N_IMG], I32)
            nc.gpsimd.iota(iota14i[:], pattern=[[1, N_IMG]], base=0, channel_multiplier=0)
            iota14f = constp.tile([1, N_IMG], F32)
            nc.vector.tensor_copy(iota14f[:], iota14i[:])
            iota64i = constp.tile([128, 64], I32)
            nc.gpsimd.iota(iota64i[:], pattern=[[1, 64]], base=0, channel_multiplier=0)
            iota64f = constp.tile([128, 64], F32)
            nc.vector.tensor_copy(iota64f[:], iota64i[:])
            iota256i = constp.tile([64, 256], I32)
            nc.gpsimd.iota(iota256i[:], pattern=[[1, 256]], base=0, channel_multiplier=0)
            iota256f = constp.tile([64, 256], F32)
            nc.vector.tensor_copy(iota256f[:], iota256i[:])
            c64 = constp.tile([128, 1], F32)
            nc.vector.memset(c64[:], 1.0 / 64.0)
            ones64_bf = constp.tile([64, 1], BF16)
            nc.gpsimd.memset(ones64_bf[:], 1.0)
            eps1 = constp.tile([1, 1], F32)
            nc.gpsimd.memset(eps1[:], 1e-30)
            onebf = workp.tile([128, 128], BF16)
            nc.gpsimd.memset(onebf[:], 1.0)
            idbf = constp.tile([128, 128], BF16)
            nc.gpsimd.affine_select(
                idbf[:],
                onebf[:],
                pattern=[[1, 128]],
                compare_op=OP.is_equal,
                fill=0.0,
                base=0,
                channel_multiplier=-1,
            )

            # ================= after xmeta arrives =================
            # ---- emoji index chain (gpsimd) -> sync register -> gather
            rmax = workp.tile([1, 1], F32)
            nc.vector.tensor_reduce(rmax[:], xrow[0:1, 5:19], AX.X, OP.max)
            dotj = workp.tile([1, N_IMG], I32)
            nc.vector.scalar_tensor_tensor(
                dotj[:], xrow[0:1, 5:19], rmax[:], iota14f[:], OP.is_ge, OP.mult
            )
            idxi = workp.tile([1, 1], I32)
            with nc.allow_low_precision(reason="argmax index sum is exact in i32"):
                nc.vector.tensor_reduce(idxi[:], dotj[:], AX.X, OP.add)
            wimg = constp.tile([S, 3 * S], BF16)
            with nc.sync.register("ridx") as ridx:
                nc.sync.reg_load(ridx, idxi[0:1, 0:1])
                off = nc.sync.snap(ridx)
                nc.sync.dma_start(
                    wimg[:], imgs_d[bass.ds(off, 1), :, :].squeeze(0)
                )

            # ---- rounded box coords cs = rint(256 * X[0:4]) (vector)
            cs = constp.tile([128, 4], F32)
            nc.vector.tensor_scalar(cs[:], xb[:, 0:4], 256.0, MAGIC, OP.mult, OP.add)
            nc.vector.tensor_scalar(cs[:], cs[:], MAGIC, None, OP.subtract)

            # ---- row exp table ErT[i, r] = exp(-(src_r[i] - (r0+r))^2), bf16
            boxr64 = constp.tile([128, 1], F32)
            nc.vector.scalar_tensor_tensor(
                boxr64[:], cs[:, 1:2], cs[:, 0:1], c64[:], OP.subtract, OP.mult
            )
            cs0r0 = workp.tile([128, 1], F32)
            nc.vector.tensor_tensor(cs0r0[:], cs[:, 0:1], xb[:, 19:20], OP.subtract)
            svecR = workp.tile([64, 1], F32)
            nc.vector.tensor_scalar(
                svecR[:], iota_pf[0:64, :], boxr64[0:64, :], cs0r0[0:64, :], OP.mult, OP.add
            )
            drT = workp.tile([64, 64], F32)
            nc.vector.tensor_scalar(drT[:], iota64f[0:64, :], -1.0, svecR[:], OP.mult, OP.add)
            drT2 = workp.tile([64, 64], F32)
            nc.scalar.square(drT2[:], drT[:])
            ErT = constp.tile([64, 64], BF16)
            nc.scalar.activation(ErT[:], drT2[:], AF.Exp, scale=-1.0)

            # ---- col exp table EcT[j, c] = exp(-(src_c[j] - c)^2), bf16
            boxc64 = constp.tile([128, 1], F32)
            nc.vector.scalar_tensor_tensor(
                boxc64[:], cs[:, 3:4], cs[:, 2:3], c64[:], OP.subtract, OP.mult
            )
            # src_c[j] grid as a row (same for every canvas column partition)
            srcc = workp.tile([128, 64], F32)
            nc.vector.tensor_scalar(
                srcc[:], iota64f[:], boxc64[:], cs[:, 2:3], OP.mult, OP.add
            )
            dc_all = workp.tile([128, 2, 64], F32)
            nc.vector.tensor_scalar(
                dc_all[:, 0, :], srcc[:], iota_pf[:], None, OP.subtract
            )
            nc.vector.tensor_scalar(
                dc_all[:, 1, :], dc_all[:, 0, :], 128.0, None, OP.subtract
            )
            dc2_all = workp.tile([128, 2, 64], F32)
            nc.scalar.square(dc2_all[:], dc_all[:])
            AcT = constp.tile([64, 256], BF16)
            for t in range(2):
                ec = workp.tile([128, 64], F32, tag="ec")
                zc = workp.tile([128, 1], F32, tag="zc")
                nc.scalar.activation(
                    ec[:], dc2_all[:, t, :], AF.Exp, scale=-1.0, accum_out=zc[:]
                )
                zc_e = workp.tile([128, 1], F32, tag="rzc")
                nc.vector.tensor_scalar(zc_e[:], zc[:], 1e-30, None, OP.add)
                nc.vector.reciprocal(zc_e[:], zc_e[:])
                Ac = workp.tile([128, 64], BF16, tag="Ac")
                nc.vector.tensor_scalar(Ac[:], ec[:], zc_e[:], None, OP.mult)
                acT_ps = ps_z.tile([64, 128], BF16, tag=f"acT{t}", name=f"acTps{t}")
                nc.tensor.transpose(acT_ps[:], Ac[:], idbf[:])
                nc.vector.tensor_copy(AcT[:, 128 * t : 128 * (t + 1)], acT_ps[:])

            # ---- row normalizer Zr via PE ones-matmul (row layout)
            zr_ps = ps_z.tile([1, 64], F32, tag="z")
            nc.tensor.matmul(zr_ps[:], ones64_bf[:], ErT[:])
            zr_row = workp.tile([1, 64], F32)
            nc.vector.tensor_scalar(zr_row[:], zr_ps[:], 1e-30, None, OP.add)

            # ---- valid flag (only the non-tautological conditions;
            # inputs are sorted in [0,1] so 0<=x1<=x2<=256 always holds)
            v4 = workp.tile([128, 1], F32)
            nc.vector.tensor_tensor(v4[:], cs[:, 1:2], cs[:, 0:1], OP.is_gt)
            v5 = workp.tile([128, 1], F32)
            nc.vector.tensor_tensor(v5[:], cs[:, 3:4], cs[:, 2:3], OP.is_gt)
            valid = constp.tile([128, 1], F32)
            nc.vector.tensor_tensor(valid[:], v4[:], v5[:], OP.mult)

            # ---- box-interval rows/cols (gpsimd) for the mask outers
            cs1r0 = workp.tile([128, 1], F32)
            nc.vector.tensor_tensor(cs1r0[:], cs[:, 1:2], xb[:, 19:20], OP.subtract)
            r_ge = workp.tile([1, 64], F32)
            nc.vector.tensor_scalar(r_ge[:], iota64f[0:1, :], cs0r0[0:1, :], None, OP.is_ge)
            r_lt = workp.tile([1, 64], F32)
            nc.vector.tensor_scalar(r_lt[:], iota64f[0:1, :], cs1r0[0:1, :], None, OP.is_lt)
            rowv2 = workp.tile([1, 128], BF16)  # valid * inside_r, stacked x2
            nc.vector.scalar_tensor_tensor(
                rowv2[0:1, 0:64], r_ge[:], valid[0:1, :], r_lt[:], OP.mult, OP.mult
            )
            nc.vector.tensor_copy(rowv2[0:1, 64:128], rowv2[0:1, 0:64])
            c_ge = workp.tile([1, 256], F32)
            nc.vector.tensor_scalar(c_ge[:], iota256f[0:1, :], cs[0:1, 2:3], None, OP.is_ge)
            c_lt = workp.tile([1, 256], F32)
            nc.vector.tensor_scalar(c_lt[:], iota256f[0:1, :], cs[0:1, 3:4], None, OP.is_lt)
            colin = workp.tile([1, 256], BF16)
            nc.vector.tensor_tensor(colin[:], c_ge[:], c_lt[:], OP.mult)

            # ---- scaled mask factor rows (bf16 for 1-pass PE outers)
            rzr_row = workp.tile([1, 64], F32)
            nc.vector.reciprocal(rzr_row[:], zr_row[:])
            rowz2 = workp.tile([1, 128], BF16)
            nc.vector.tensor_tensor(rowz2[0:1, 0:64], rowv2[0:1, 0:64], rzr_row[:], OP.mult)
            nc.vector.tensor_copy(rowz2[0:1, 64:128], rowz2[0:1, 0:64])

            # ---- mask outer products on PE
            mz_ps = ps_m.tile([128, 256], F32, tag="mz", name="mzps")
            nc.tensor.matmul(mz_ps[:], rowz2[:], colin[:])
            mv_ps = ps_m.tile([128, 256], F32, tag="mv", name="mvps")
            nc.tensor.matmul(mv_ps[:], rowv2[:], colin[:])
            mz_sb = constp.tile([128, 256], F32)
            nc.scalar.copy(mz_sb[:], mz_ps[:])
            w2 = constp.tile([128, 256], F32)
            nc.vector.tensor_scalar(w2[:], mv_ps[:], -1.0, valid[:], OP.mult, OP.add)

            # ---- t1[ch][j, r] = sum_i wimg[i, (ch,j)] * ErT[i, r]
            t1_ps = ps_t1.tile([64, 192], F32, tag="t1")
            for ch in range(3):
                nc.tensor.matmul(
                    t1_ps[:, 64 * ch : 64 * (ch + 1)],
                    wimg[:, 64 * ch : 64 * (ch + 1)],
                    ErT[:],
                )
            t1all = constp.tile([64, 192], BF16)
            nc.scalar.copy(t1all[:, 0:64], t1_ps[:, 0:64])
            nc.scalar.copy(t1all[:, 64:128], t1_ps[:, 64:128])
            nc.vector.tensor_copy(t1all[:, 128:192], t1_ps[:, 128:192])

            # ---- eR matmuls
            er_ab_ps = ps_r.tile([128, 256], F32, tag="rab", name="erab")
            nc.tensor.matmul(er_ab_ps[:], t1all[:, 0:128], AcT[:])
            er_c_ps = ps_r.tile([64, 256], F32, tag="rc", name="erc")
            nc.tensor.matmul(er_c_ps[:], t1all[:, 128:192], AcT[:])

            # ---- blend: res = eR*Mz + (valid - Mv)
            res_ab = outp.tile([128, 256], F32)
            nc.vector.tensor_tensor(res_ab[:], er_ab_ps[:], mz_sb[:], OP.mult)
            nc.vector.tensor_tensor(res_ab[:], res_ab[:], w2[:], OP.add)
            res_c = outp.tile([64, 256], F32)
            nc.vector.tensor_tensor(res_c[:], er_c_ps[:], mz_sb[0:64, :], OP.mult)
            nc.vector.tensor_tensor(res_c[:], res_c[:], w2[0:64, :], OP.add)

            # ---- output DMAs on the two HWDGE rings in parallel
            nc.sync.dma_start(
                out_d[0:2, :, :].rearrange("a b c -> (a b) c"), res_ab[:]
            )
            nc.scalar.dma_start(out_d[2, :, :], res_c[:])

    nc.compile()
    return nc


_CACHE = {}


def get_nc():
    if "nc" not in _CACHE:
        _CACHE["nc"] = build_nc()
    return _CACHE["nc"]


def make_in_maps(X, images):
    X = np.ascontiguousarray(np.asarray(X, np.float32))
    images = np.ascontiguousarray(np.asarray(images, np.float32))
    # layout/dtype prep only: [14,4,64,64] f32 -> [14, 64(i), 3*64(ch,j)] bf16
    imgs_gt = np.ascontiguousarray(
        images[:, 0:3].transpose(0, 2, 1, 3).reshape(N_IMG, S, 3 * S)
    ).astype(ml_dtypes.bfloat16)
    in_maps = []
    for c in range(N_CORES):
        pic, rb = divmod(c, 4)
        xm = np.zeros((1, 24), np.float32)
        xm[0, :19] = X[pic, 0]
        xm[0, 19] = float(RB * rb)
        in_maps.append(
            {"xmeta": np.tile(xm, (128, 1)), "xrow": xm.copy(), "imgs": imgs_gt}
        )
    return in_maps


def assemble(results):
    out = np.empty((2, 3, H, H), np.float32)
    for c in range(N_CORES):
        pic, rb = divmod(c, 4)
        out[pic, :, RB * rb : RB * (rb + 1), :] = results[c]["out"]
    return out


def _axon_reset():
    try:
        import ctypes

        import jax

        jax.devices()
        ctypes.CDLL("/opt/axon/libaxon_pjrt.so").axon_reset()
    except Exception:
        pass


def kernel(X, images):
    nc = get_nc()
    in_maps = make_in_maps(X, images)
    try:
        res = run_bass_kernel_spmd(nc, in_maps, list(range(N_CORES)))
    except Exception:
        # the axon terminal can be left in a bad state by earlier failed
        # runs (LoadExecutable errors); reset and retry once
        _axon_reset()
        res = run_bass_kernel_spmd(nc, in_maps, list(range(N_CORES)))
    return assemble(res.results)


# revision 22
# speedup vs baseline: 1.3596x; 1.0251x over previous
"""Trainium2 Bass kernel for the emoji-box decoder problem (optimized v2).

Math: per picture, softmax(-d2) over emoji pixels is separable:
softmax_r (x) softmax_c.  This version postpones BOTH softmax
normalizations: it computes unnormalized ErT[i,r] = exp(-(src_r[i]-r)^2)
and EcT[j,c] = exp(-(src_c[j]-c)^2) DIRECTLY in transposed layout
(partition = emoji index), so no PE transposes are needed.  The
row normalizer Zr = sum_i ErT comes from a PE ones-matmul and 1/Zr is
folded into the box mask outer product.  The column softmax Ac is
computed in natural [c, j] layout (fast per-partition normalize) and
PE-transposed to AcT:
    res = eR * Mz + (valid - Mv)
    eR  = t1[j,(ch,r)] = sum_i wimg[i,(ch,j)] ErT[i,r],
          eR[(ch,r),c] = sum_j t1 AcT[j,c]
    Mz  = (valid*rowin*rzr) (x) colin,  Mv = (valid*rowin) (x) colin
which equals where(valid, where(inside, R, 1), 0).

Emoji selection: argmax of logits -> gpsimd register -> dynamic-offset
DRAM->SBUF DMA gather of the selected emoji (bf16, i-major layout so
each partition reads one contiguous 384B chunk).

Matmuls run in bf16 (1-pass PE) — images are cast to bf16 on the host
(layout/dtype prep only), exp outputs are written as bf16.

Sharding: 8 cores = 2 pictures x 4 row-blocks of 64 canvas rows.
xmeta is host-replicated to all 128 partitions (no on-device broadcast).
"""

import sys

import numpy as np

if "/opt/trn_rl_repo" not in sys.path:
    sys.path.insert(0, "/opt/trn_rl_repo")

import ml_dtypes

import concourse.bacc as bacc
import concourse.bass as bass
import concourse.mybir as mybir
import concourse.tile as tile
from concourse.bass_utils import run_bass_kernel_spmd


def _ensure_ntff_hook():
    """The image's antenv package lacks axon_hooks, so trn_boot's NTFF
    profile hook install degrades silently and run_bass_kernel_spmd
    crashes on `from antenv.axon_hooks import ...` when trace=True.
    Provide the module and install the ctypes hook ourselves."""
    import types

    try:
        from antenv.axon_hooks import get_axon_ntff_profile_hook  # noqa: F401

        return
    except ImportError:
        pass
    mod = types.ModuleType("antenv.axon_hooks")
    _hook = [None]
    mod.set_axon_ntff_profile_hook = lambda h: _hook.__setitem__(0, h)
    mod.get_axon_ntff_profile_hook = lambda: _hook[0]
    try:
        import antenv

        sys.modules["antenv.axon_hooks"] = mod
        antenv.axon_hooks = mod
        from trn_agent_boot.trn_boot import _ntff_profile_via_ctypes

        hook = _ntff_profile_via_ctypes("/opt/axon/libaxon_pjrt.so")
        if hook is not None:
            mod.set_axon_ntff_profile_hook(hook)
    except Exception:
        pass


_ensure_ntff_hook()

F32 = mybir.dt.float32
BF16 = mybir.dt.bfloat16
I32 = mybir.dt.int32
AF = mybir.ActivationFunctionType
OP = mybir.AluOpType
AX = mybir.AxisListType

MAGIC = 8388608.0  # 2**23; x + MAGIC - MAGIC == rint(x) for 0 <= x < 2**22

N_CORES = 8
H = 256
S = 64
N_IMG = 14
RB = 64  # canvas rows per core


def build_nc():
    nc = bacc.Bacc("TRN2", target_bir_lowering=False, debug=False)

    xmeta_d = nc.dram_tensor("xmeta", [128, 24], F32, kind="ExternalInput")
    xrow_d = nc.dram_tensor("xrow", [1, 24], F32, kind="ExternalInput")
    imgs_d = nc.dram_tensor("imgs", [N_IMG, S, 3 * S], BF16, kind="ExternalInput")
    out_d = nc.dram_tensor("out", [3, RB, H], F32, kind="ExternalOutput")

    with tile.TileContext(nc) as tc:
        with (
            tc.tile_pool(name="constp", bufs=1) as constp,
            tc.tile_pool(name="workp", bufs=2) as workp,
            tc.tile_pool(name="outp", bufs=1) as outp,
            tc.tile_pool(name="ps_z", bufs=1, space="PSUM") as ps_z,
            tc.tile_pool(name="ps_m", bufs=1, space="PSUM") as ps_m,
            tc.tile_pool(name="ps_t1", bufs=1, space="PSUM") as ps_t1,
            tc.tile_pool(name="ps_r", bufs=1, space="PSUM") as ps_r,
        ):
            # ---- input DMAs first: xrow (sync ring, 1 descriptor -> fast
            # round trip, feeds the argmax/gather chain) and the broadcast
            # copy xb (scalar ring, emitted before the warm activation so
            # the DMA issue precedes the ACT_TABLE_LOAD on that queue)
            xrow = constp.tile([1, 24], F32)
            nc.sync.dma_start(xrow[:], xrow_d[:])
            xb = constp.tile([128, 24], F32)
            nc.scalar.dma_start(xb[:], xmeta_d[:])

            # ---- warm the scalar-engine activation table early so the
            # ~1.3us ACT_TABLE_LOAD overlaps the input DMAs
            warm = workp.tile([1, 1], F32)
            nc.gpsimd.memset(warm[:], 0.0)
            warm2 = workp.tile([1, 1], F32)
            nc.scalar.activation(warm2[:], warm[:], AF.Exp)

            # ---- constants / iotas (gpsimd+vector, overlap the DMAs)
            iota_pi = constp.tile([128, 1], I32)
            nc.gpsimd.iota(iota_pi[:], pattern=[[1, 1]], base=0, channel_multiplier=1)
            iota_pf = constp.tile([128, 1], F32)
            nc.vector.tensor_copy(iota_pf[:], iota_pi[:])
            iota14i = constp.tile([1, N_IMG], I32)
            nc.gpsimd.iota(iota14i[:], pattern=[[1, N_IMG]], base=0, channel_multiplier=0)
            iota14f = constp.tile([1, N_IMG], F32)
            nc.vector.tensor_copy(iota14f[:], iota14i[:])
            iota64i = constp.tile([128, 64], I32)
            nc.gpsimd.iota(iota64i[:], pattern=[[1, 64]], base=0, channel_multiplier=0)
            iota64f = constp.tile([128, 64], F32)
            nc.vector.tensor_copy(iota64f[:], iota64i[:])
            iota256i = constp.tile([64, 256], I32)
            nc.gpsimd.iota(iota256i[:], pattern=[[1, 256]], base=0, channel_multiplier=0)
            iota256f = constp.tile([64, 256], F32)
            nc.vector.tensor_copy(iota256f[:], iota256i[:])
            c64 = constp.tile([128, 1], F32)
            nc.vector.memset(c64[:], 1.0 / 64.0)
            ones64_bf = constp.tile([64, 1], BF16)
            nc.gpsimd.memset(ones64_bf[:], 1.0)
            eps1 = constp.tile([1, 1], F32)
            nc.gpsimd.memset(eps1[:], 1e-30)
            onebf = workp.tile([128, 128], BF16)
            nc.gpsimd.memset(onebf[:], 1.0)
            idbf = constp.tile([128, 128], BF16)
            nc.gpsimd.affine_select(
                idbf[:],
                onebf[:],
                pattern=[[1, 128]],
                compare_op=OP.is_equal,
                fill=0.0,
                base=0,
                channel_multiplier=-1,
            )

            # ================= after xmeta arrives =================
            # ---- emoji index chain (gpsimd) -> sync register -> gather
            rmax = workp.tile([1, 1], F32)
            nc.vector.tensor_reduce(rmax[:], xrow[0:1, 5:19], AX.X, OP.max)
            dotj = workp.tile([1, N_IMG], I32)
            nc.vector.scalar_tensor_tensor(
                dotj[:], xrow[0:1, 5:19], rmax[:], iota14f[:], OP.is_ge, OP.mult
            )
            idxi = workp.tile([1, 1], I32)
            with nc.allow_low_precision(reason="argmax index sum is exact in i32"):
                nc.vector.tensor_reduce(idxi[:], dotj[:], AX.X, OP.add)
            wimg = constp.tile([S, 3 * S], BF16)
            with nc.sync.register("ridx") as ridx:
                nc.sync.reg_load(ridx, idxi[0:1, 0:1])
                off = nc.sync.snap(ridx)
                nc.sync.dma_start(
                    wimg[:], imgs_d[bass.ds(off, 1), :, :].squeeze(0)
                )

            # ---- rounded box coords cs = rint(256 * X[0:4]) (vector)
            cs = constp.tile([128, 4], F32)
            nc.vector.tensor_scalar(cs[:], xb[:, 0:4], 256.0, MAGIC, OP.mult, OP.add)
            nc.vector.tensor_scalar(cs[:], cs[:], MAGIC, None, OP.subtract)

            # ---- row exp table ErT[i, r] = exp(-(src_r[i] - (r0+r))^2), bf16
            boxr64 = constp.tile([128, 1], F32)
            nc.vector.scalar_tensor_tensor(
                boxr64[:], cs[:, 1:2], cs[:, 0:1], c64[:], OP.subtract, OP.mult
            )
            cs0r0 = workp.tile([128, 1], F32)
            nc.vector.tensor_tensor(cs0r0[:], cs[:, 0:1], xb[:, 19:20], OP.subtract)
            svecR = workp.tile([64, 1], F32)
            nc.vector.tensor_scalar(
                svecR[:], iota_pf[0:64, :], boxr64[0:64, :], cs0r0[0:64, :], OP.mult, OP.add
            )
            drT = workp.tile([64, 64], F32)
            nc.vector.tensor_scalar(drT[:], iota64f[0:64, :], -1.0, svecR[:], OP.mult, OP.add)
            drT2 = workp.tile([64, 64], F32)
            nc.scalar.square(drT2[:], drT[:])
            ErT = constp.tile([64, 64], BF16)
            nc.scalar.activation(ErT[:], drT2[:], AF.Exp, scale=-1.0)

            # ---- col exp table EcT[j, c] = exp(-(src_c[j] - c)^2), bf16
            boxc64 = constp.tile([128, 1], F32)
            nc.vector.scalar_tensor_tensor(
                boxc64[:], cs[:, 3:4], cs[:, 2:3], c64[:], OP.subtract, OP.mult
            )
            # src_c[j] grid as a row (same for every canvas column partition)
            srcc = workp.tile([128, 64], F32)
            nc.vector.tensor_scalar(
                srcc[:], iota64f[:], boxc64[:], cs[:, 2:3], OP.mult, OP.add
            )
            dc_all = workp.tile([128, 2, 64], F32)
            nc.vector.tensor_scalar(
                dc_all[:, 0, :], srcc[:], iota_pf[:], None, OP.subtract
            )
            nc.vector.tensor_scalar(
                dc_all[:, 1, :], dc_all[:, 0, :], 128.0, None, OP.subtract
            )
            dc2_all = workp.tile([128, 2, 64], F32)
            nc.scalar.square(dc2_all[:], dc_all[:])
            # ---- valid flag (only the non-tautological conditions;
            # inputs are sorted in [0,1] so 0<=x1<=x2<=256 always holds)
            v4 = workp.tile([128, 1], F32)
            nc.vector.tensor_tensor(v4[:], cs[:, 1:2], cs[:, 0:1], OP.is_gt)
            v5 = workp.tile([128, 1], F32)
            nc.vector.tensor_tensor(v5[:], cs[:, 3:4], cs[:, 2:3], OP.is_gt)
            valid = constp.tile([128, 1], F32)
            nc.vector.tensor_tensor(valid[:], v4[:], v5[:], OP.mult)

            # ---- box-interval rows/cols (gpsimd) for the mask outers
            cs1r0 = workp.tile([128, 1], F32)
            nc.vector.tensor_tensor(cs1r0[:], cs[:, 1:2], xb[:, 19:20], OP.subtract)
            r_ge = workp.tile([1, 64], F32)
            nc.vector.tensor_scalar(r_ge[:], iota64f[0:1, :], cs0r0[0:1, :], None, OP.is_ge)
            r_lt = workp.tile([1, 64], F32)
            nc.vector.tensor_scalar(r_lt[:], iota64f[0:1, :], cs1r0[0:1, :], None, OP.is_lt)
            rowv2 = workp.tile([1, 128], BF16)  # valid * inside_r, stacked x2
            nc.vector.scalar_tensor_tensor(
                rowv2[0:1, 0:64], r_ge[:], valid[0:1, :], r_lt[:], OP.mult, OP.mult
            )
            nc.vector.tensor_copy(rowv2[0:1, 64:128], rowv2[0:1, 0:64])
            c_ge = workp.tile([1, 256], F32)
            nc.vector.tensor_scalar(c_ge[:], iota256f[0:1, :], cs[0:1, 2:3], None, OP.is_ge)
            c_lt = workp.tile([1, 256], F32)
            nc.vector.tensor_scalar(c_lt[:], iota256f[0:1, :], cs[0:1, 3:4], None, OP.is_lt)
            colin = workp.tile([1, 256], BF16)
            nc.vector.tensor_tensor(colin[:], c_ge[:], c_lt[:], OP.mult)

            AcT = constp.tile([64, 256], BF16)
            for t in range(2):
                ec = workp.tile([128, 64], F32, tag="ec")
                zc = workp.tile([128, 1], F32, tag="zc")
                nc.scalar.activation(
                    ec[:], dc2_all[:, t, :], AF.Exp, scale=-1.0, accum_out=zc[:]
                )
                zc_e = workp.tile([128, 1], F32, tag="rzc")
                nc.vector.tensor_scalar(zc_e[:], zc[:], 1e-30, None, OP.add)
                nc.vector.reciprocal(zc_e[:], zc_e[:])
                Ac = workp.tile([128, 64], BF16, tag="Ac")
                nc.vector.tensor_scalar(Ac[:], ec[:], zc_e[:], None, OP.mult)
                acT_ps = ps_z.tile([64, 128], BF16, tag=f"acT{t}", name=f"acTps{t}")
                nc.tensor.transpose(acT_ps[:], Ac[:], idbf[:])
                nc.vector.tensor_copy(AcT[:, 128 * t : 128 * (t + 1)], acT_ps[:])

            # ---- row normalizer Zr via PE ones-matmul (row layout)
            zr_ps = ps_z.tile([1, 64], F32, tag="z")
            nc.tensor.matmul(zr_ps[:], ones64_bf[:], ErT[:])
            zr_row = workp.tile([1, 64], F32)
            nc.vector.tensor_scalar(zr_row[:], zr_ps[:], 1e-30, None, OP.add)

            # ---- scaled mask factor rows (bf16 for 1-pass PE outers)
            rzr_row = workp.tile([1, 64], F32)
            nc.vector.reciprocal(rzr_row[:], zr_row[:])
            rowz2 = workp.tile([1, 128], BF16)
            nc.vector.tensor_tensor(rowz2[0:1, 0:64], rowv2[0:1, 0:64], rzr_row[:], OP.mult)
            nc.vector.tensor_copy(rowz2[0:1, 64:128], rowz2[0:1, 0:64])

            # ---- mask outer products on PE
            mz_ps = ps_m.tile([128, 256], F32, tag="mz", name="mzps")
            nc.tensor.matmul(mz_ps[:], rowz2[:], colin[:])
            mv_ps = ps_m.tile([128, 256], F32, tag="mv", name="mvps")
            nc.tensor.matmul(mv_ps[:], rowv2[:], colin[:])
            # ---- t1[ch][j, r] = sum_i wimg[i, (ch,j)] * ErT[i, r]
            t1_ps = ps_t1.tile([64, 192], F32, tag="t1")
            for ch in range(3):
                nc.tensor.matmul(
                    t1_ps[:, 64 * ch : 64 * (ch + 1)],
                    wimg[:, 64 * ch : 64 * (ch + 1)],
                    ErT[:],
                )
            t1all = constp.tile([64, 192], BF16)
            nc.scalar.copy(t1all[:, 0:64], t1_ps[:, 0:64])
            nc.scalar.copy(t1all[:, 64:128], t1_ps[:, 64:128])
            nc.vector.tensor_copy(t1all[:, 128:192], t1_ps[:, 128:192])

            mz_sb = constp.tile([128, 256], F32)
            nc.scalar.copy(mz_sb[:], mz_ps[:])
            w2 = constp.tile([128, 256], F32)
            nc.vector.tensor_scalar(w2[:], mv_ps[:], -1.0, valid[:], OP.mult, OP.add)


            # ---- eR matmuls
            er_ab_ps = ps_r.tile([128, 256], F32, tag="rab", name="erab")
            nc.tensor.matmul(er_ab_ps[:], t1all[:, 0:128], AcT[:])
            er_c_ps = ps_r.tile([64, 256], F32, tag="rc", name="erc")
            nc.tensor.matmul(er_c_ps[:], t1all[:, 128:192], AcT[:])

            # ---- blend: res = eR*Mz + (valid - Mv)
            res_ab = outp.tile([128, 256], F32)
            nc.vector.tensor_tensor(res_ab[:], er_ab_ps[:], mz_sb[:], OP.mult)
            nc.vector.tensor_tensor(res_ab[:], res_ab[:], w2[:], OP.add)
            res_c = outp.tile([64, 256], F32)
            nc.vector.tensor_tensor(res_c[:], er_c_ps[:], mz_sb[0:64, :], OP.mult)
            nc.vector.tensor_tensor(res_c[:], res_c[:], w2[0:64, :], OP.add)

            # ---- output DMAs on the two HWDGE rings in parallel
            nc.sync.dma_start(
                out_d[0:2, :, :].rearrange("a b c -> (a b) c"), res_ab[:]
            )
            nc.scalar.dma_start(out_d[2, :, :], res_c[:])

    nc.compile()
    return nc


_CACHE = {}


def get_nc():
    if "nc" not in _CACHE:
        _CACHE["nc"] = build_nc()
    return _CACHE["nc"]


def make_in_maps(X, images):
    X = np.ascontiguousarray(np.asarray(X, np.float32))
    images = np.ascontiguousarray(np.asarray(images, np.float32))
    # layout/dtype prep only: [14,4,64,64] f32 -> [14, 64(i), 3*64(ch,j)] bf16
    imgs_gt = np.ascontiguousarray(
        images[:, 0:3].transpose(0, 2, 1, 3).reshape(N_IMG, S, 3 * S)
    ).astype(ml_dtypes.bfloat16)
    in_maps = []
    for c in range(N_CORES):
        pic, rb = divmod(c, 4)
        xm = np.zeros((1, 24), np.float32)
        xm[0, :19] = X[pic, 0]
        xm[0, 19] = float(RB * rb)
        in_maps.append(
            {"xmeta": np.tile(xm, (128, 1)), "xrow": xm.copy(), "imgs": imgs_gt}
        )
    return in_maps


def assemble(results):
    out = np.empty((2, 3, H, H), np.float32)
    for c in range(N_CORES):
        pic, rb = divmod(c, 4)
        out[pic, :, RB * rb : RB * (rb + 1), :] = results[c]["out"]
    return out


def _axon_reset():
    try:
        import ctypes

        import jax

        jax.devices()
        ctypes.CDLL("/opt/axon/libaxon_pjrt.so").axon_reset()
    except Exception:
        pass


def kernel(X, images):
    nc = get_nc()
    in_maps = make_in_maps(X, images)
    try:
        res = run_bass_kernel_spmd(nc, in_maps, list(range(N_CORES)))
    except Exception:
        # the axon terminal can be left in a bad state by earlier failed
        # runs (LoadExecutable errors); reset and retry once
        _axon_reset()
        res = run_bass_kernel_spmd(nc, in_maps, list(range(N_CORES)))
    return assemble(res.results)
